# revision 1
# baseline (speedup 1.0000x reference)
"""Trainium2 Bass kernel for nn_GAT_GCN (gnn_message_passing), 8 NeuronCores.

Strategy:
 - Shard destination nodes across the 8 cores, aligned to graph boundaries
   (16 graphs/core), edges pre-sorted by dst on the host.
 - Segment softmax/scatter-add are done as matmuls against host-built 0/1
   selection blocks (S01); GCN's symmetric norm is folded into the S values.
 - Per-edge gathers use dma_gather (int16 idxs, 256B-multiple fp16 rows).
 - GAT1/GCN1 gather raw x (transposed) and project on the fly; GAT2 gathers a
   pre-projected [h2 | asrc2] table that is exchanged with one AllGather.
 - Softmax skips the segment-max shift (mathematically identical, logits tiny).
"""
import sys
sys.path.insert(0, '/opt/trn_rl_repo')
import numpy as np

N, E, G, F, H = 16384, 131072, 128, 78, 10
NCORE, GPC = 8, 16          # cores, graphs per core
HF = H * F                  # 780
WAUG1 = HF + F + H          # 868 = h1(780) | hgcn(78) | asrc(10)
T2W = HF + H                # 790 = h2 | asrc2
W2AUG = HF + 2 * H          # 800 = h2 | asrc2 | adst2
TROW = 896                  # T2/x1f table row, fp16 (1792B, %256)
XROW = 128                  # x / x2 table row, fp16 (256B)


def _wrap16(v):
    """dma_gather idx layout: [128, len/16] int16, idx i at (i%16, i//16),
    replicated across the 8 Q7 core groups."""
    v = np.asarray(v, np.int16)
    assert len(v) % 16 == 0
    m = v.reshape(-1, 16).T            # [16, S]
    return np.tile(m, (8, 1)).copy()   # [128, S]


def _f16(a):
    return np.ascontiguousarray(np.asarray(a, np.float32)).astype(np.float16)


def prep(x, edge_index, batch, target, Wg1, as1, ad1, bg1, Wg2, as2, ad2, bg2,
         Wgcn, bgcn, Wfg1, bfg1, Wfg2, bfg2, wconv, bconv, Wxt, bxt,
         W1, b1, W2, b2, Wo, bo):
    x = np.asarray(x, np.float32)
    ei = np.asarray(edge_index, np.int64)
    batch = np.asarray(batch, np.int64)
    target = np.asarray(target, np.float32)

    loops = np.arange(N, dtype=np.int64)
    src = np.concatenate([ei[0], loops])
    dst = np.concatenate([ei[1], loops])

    # graph-aligned core boundaries
    counts = np.bincount(batch, minlength=G)
    node_off = np.concatenate([[0], np.cumsum(counts)])
    n_lo = node_off[np.arange(NCORE) * GPC]
    n_hi = node_off[(np.arange(NCORE) + 1) * GPC]

    # degrees / gcn norm (over full edge list incl self loops)
    deg = np.bincount(dst, minlength=N).astype(np.float64)
    dinv = 1.0 / np.sqrt(deg)
    norm = (dinv[src] * dinv[dst]).astype(np.float32)

    order = np.argsort(dst, kind='stable')
    srcs, dsts, norms = src[order], dst[order], norm[order]

    Lmax = int((n_hi - n_lo).max())
    NBLK = (Lmax + 127) // 128
    NPC = NBLK * 128
    assert NCORE * NPC < 32768

    owner = np.searchsorted(n_hi, src, side='right')   # owner core of each node id? (by src value)
    node_owner = np.searchsorted(n_hi - 1, np.arange(N), side='left')
    node_owner = np.minimum(node_owner, NCORE - 1)
    # padded global id used for allgathered tables
    pad_gid = node_owner * NPC + (np.arange(N) - n_lo[node_owner])

    # per (core, block) edge spans -> uniform TPB
    spans = []
    TPB = 1
    for c in range(NCORE):
        e0 = np.searchsorted(dsts, n_lo[c])
        e1 = np.searchsorted(dsts, n_hi[c])
        bl = []
        for b in range(NBLK):
            lo = np.searchsorted(dsts, n_lo[c] + 128 * b)
            hi = np.searchsorted(dsts, min(n_lo[c] + 128 * (b + 1), n_hi[c]))
            if n_lo[c] + 128 * b >= n_hi[c]:
                lo = hi = e1
            bl.append((lo, hi))
            TPB = max(TPB, (hi - lo + 127) // 128)
        spans.append(bl)
    ET = NBLK * TPB
    ECAP = ET * 128

    PW = int(np.ceil(counts.max() / 16) * 16)   # pool slot width per graph

    cores = []
    for c in range(NCORE):
        esrc = np.zeros(ECAP, np.int64)           # raw src id per edge slot
        s01 = np.zeros((ET, 128, 128), np.float16)
        snrm = np.zeros((ET, 128, 128), np.float16)
        s01t = np.zeros((NBLK, 128, TPB * 128), np.float16)
        for b in range(NBLK):
            lo, hi = spans[c][b]
            ne = hi - lo
            if ne == 0:
                continue
            sl = slice(b * TPB * 128, b * TPB * 128 + ne)
            esrc[sl] = srcs[lo:hi]
            ld = (dsts[lo:hi] - n_lo[c] - 128 * b).astype(np.int64)  # 0..127
            j = np.arange(ne)
            t_loc = j // 128
            e_loc = j % 128
            s01[b * TPB + t_loc, e_loc, ld] = 1.0
            snrm[b * TPB + t_loc, e_loc, ld] = norms[lo:hi].astype(np.float16)
            s01t[b, ld, j] = 1.0
        # padded dst columns (no incoming edges) get one dummy S entry so the
        # softmax denominator stays finite (their rows are garbage, never read)
        Lc = int(n_hi[c] - n_lo[c])
        for b in range(NBLK):
            first_pad = max(0, min(128, Lc - 128 * b))
            if first_pad < 128:
                s01[b * TPB, 0, first_pad:] = 1.0

        # pooling indices (local node ids into x1f/x2f tables)
        pool_idx = np.zeros(GPC * PW, np.int64)
        for g in range(GPC):
            gg = c * GPC + g
            a, bnd = node_off[gg] - n_lo[c], node_off[gg + 1] - n_lo[c]
            cnt = bnd - a
            pool_idx[g * PW:g * PW + cnt] = np.arange(a, bnd)
            pool_idx[g * PW + cnt:(g + 1) * PW] = a      # pad = first node of graph
        mmean = np.zeros((NBLK, 128, GPC), np.float16)
        for g in range(GPC):
            gg = c * GPC + g
            a, bnd = node_off[gg] - n_lo[c], node_off[gg + 1] - n_lo[c]
            ids = np.arange(a, bnd)
            mmean[ids // 128, ids % 128, g] = np.float16(1.0 / (bnd - a))

        # conv im2col: [32, GPC, 608]
        t_win = np.zeros((32, GPC, 608), np.float16)
        tg = target[c * GPC:(c + 1) * GPC, 0, :]          # [GPC, 625]
        for k in range(32):
            t_win[k, :, :594] = tg[:, k:k + 594].astype(np.float16)

        L = int(n_hi[c] - n_lo[c])
        xT = np.zeros((128, NPC), np.float16)
        xT[:F, :L] = x[n_lo[c]:n_hi[c], :].T.astype(np.float16)

        s_comb = np.zeros((NBLK, 128, TPB * 256), np.float16)
        for b in range(NBLK):
            for k in range(TPB):
                s_comb[b, :, k * 256:k * 256 + 128] = s01[b * TPB + k]
                s_comb[b, :, k * 256 + 128:k * 256 + 256] = snrm[b * TPB + k]
        cores.append(dict(
            ix_x=_wrap16(esrc),                     # for x-gather (raw ids)
            ix_t2=_wrap16(pad_gid[esrc]),           # for T2/x2 gathers (padded ids)
            ix_pool=_wrap16(pool_idx),
            s01=s01, snrm=snrm, s01t=s01t, s_comb=s_comb,
            mmean=mmean, t_win=t_win, xT_loc=xT,
            bconv_rep=np.full((GPC, 1), float(bconv[0]), np.float32),
        ))

    x16 = np.zeros((N, XROW), np.float16)
    x16[:, :F] = x.astype(np.float16)

    Wg1cat = np.zeros((128, WAUG1), np.float16)
    Wg1cat[:F, :HF] = _f16(Wg1)
    Wg1cat[:F, HF:HF + F] = _f16(Wgcn)
    W2chunks = np.zeros((7, 128, W2AUG), np.float16)
    for k in range(7):
        r0, r1 = 128 * k, min(128 * (k + 1), HF)
        W2chunks[k, :r1 - r0, :HF] = _f16(Wg2[r0:r1, :])
    bg1ch = np.zeros((7, 128, 1), np.float16)
    bg1f = _f16(bg1).reshape(-1)
    for k in range(7):
        r0, r1 = 128 * k, min(128 * (k + 1), HF)
        bg1ch[k, :r1 - r0, 0] = bg1f[r0:r1]

    def pack_rows(Wm, splits, ncol):
        out = np.zeros((len(splits), 128, ncol), np.float16)
        for i, (r0, r1) in enumerate(splits):
            out[i, :r1 - r0, :] = _f16(Wm[r0:r1, :])
        return out

    sp7 = [(128 * i, min(128 * (i + 1), HF)) for i in range(7)]
    wfg1p = np.concatenate([pack_rows(Wfg1[:HF], sp7, 128),
                            pack_rows(Wfg1[HF:], sp7, 128)], axis=0)  # [14,128,128]
    wfg2p = pack_rows(Wfg2, [(0, F), (F, 2 * F)], 128)                # [2,128,128]
    wxtp = pack_rows(Wxt, [(128 * i, min(128 * (i + 1), 594)) for i in range(5)], 256)
    w1p = pack_rows(W1, [(128 * i, 128 * (i + 1)) for i in range(4)], 512)
    w2p = pack_rows(W2, [(128 * i, 128 * (i + 1)) for i in range(4)], 256)
    wop = pack_rows(Wo, [(0, 128), (128, 256)], 1)

    wgcn_s = np.zeros((128, F), np.float16)
    wgcn_s[:F] = _f16(Wgcn)
    bgcn_col = np.zeros((128, 1), np.float32)
    bgcn_col[:F, 0] = np.asarray(bgcn, np.float32)

    shared = dict(
        x16=x16, Wg1cat=Wg1cat, W2chunks=W2chunks, bg1ch=bg1ch,
        as1f=_f16(as1).reshape(1, HF), ad1f=_f16(ad1).reshape(1, HF),
        as2f=_f16(as2).reshape(1, HF), ad2f=_f16(ad2).reshape(1, HF),
        wgcn_s=wgcn_s, bgcn_col=bgcn_col,
        bgcn_row=np.asarray(bgcn, np.float32).reshape(1, F),
        bg2row=np.asarray(bg2, np.float32).reshape(1, HF),
        wfg1p=wfg1p, bfg1=np.asarray(bfg1, np.float32).reshape(1, 128),
        wfg2p=wfg2p, bfg2=np.asarray(bfg2, np.float32).reshape(1, 128),
        wxtp=wxtp, bxt=np.asarray(bxt, np.float32).reshape(1, 256),
        w1p=w1p, b1=np.asarray(b1, np.float32).reshape(1, 512),
        w2p=w2p, b2=np.asarray(b2, np.float32).reshape(1, 256),
        wop=wop, bo_rep=np.full((GPC, 1), float(np.asarray(bo).reshape(-1)[0]), np.float32),
        w_col=np.zeros((32, 1), np.float16),
        w_sel=np.zeros((32, GPC, GPC), np.float16),
    )
    shared['w_col'][:, 0] = _f16(np.asarray(wconv).reshape(-1))
    for g in range(GPC):
        shared['w_sel'][:, g, g] = shared['w_col'][:, 0]

    meta = dict(NBLK=NBLK, NPC=NPC, TPB=TPB, ET=ET, ECAP=ECAP, PW=PW,
                n_lo=n_lo, n_hi=n_hi)
    return meta, shared, cores


# ---------------------------------------------------------------- numpy sim

def unwrap16(m):
    """inverse of _wrap16: [128, S] -> [S*16] (first 16-partition group)."""
    return np.asarray(m[:16, :].T.reshape(-1), np.int64)

import concourse.bass as bass
import concourse.bacc as bacc
import concourse.mybir as mybir
from concourse import library_config
from concourse.tile import TileContext
from concourse.masks import make_identity
from concourse.bass_utils import run_bass_kernel_spmd

F16 = mybir.dt.float16
F32 = mybir.dt.float32
I16 = mybir.dt.int16
AX = mybir.AxisListType.X
ALU = mybir.AluOpType
AF = mybir.ActivationFunctionType



def build(meta):
    NBLK, NPC, TPB, ET, ECAP, PW = (meta[k] for k in
                                    ['NBLK', 'NPC', 'TPB', 'ET', 'ECAP', 'PW'])
    EPB = TPB * 128                       # edges per block
    nc = bacc.Bacc()

    dp = lambda n, s, d: nc.declare_dram_parameter(n, list(s), d, isOutput=False)
    # per-core inputs
    x16 = dp('x16', [N, XROW], F16)
    xT_loc = dp('xT_loc', [128, NPC], F16)
    ix_x = dp('ix_x', [128, ECAP // 16], I16)
    ix_t2 = dp('ix_t2', [128, ECAP // 16], I16)
    ix_pool = dp('ix_pool', [128, GPC * PW // 16], I16)
    scomb_d = dp('s_comb', [NBLK, 128, TPB * 256], F16)
    s01t_d = dp('s01t', [NBLK, 128, EPB], F16)
    mmean_d = dp('mmean', [NBLK, 128, GPC], F16)
    twin_d = dp('t_win', [32, GPC, 608], F16)
    bconv_rep = dp('bconv_rep', [GPC, 1], F32)
    # shared weights
    wg1cat = dp('Wg1cat', [128, WAUG1], F16)
    w2ch = dp('W2chunks', [7, 128, W2AUG], F16)
    bg1ch = dp('bg1ch', [7, 128, 1], F16)
    as1f, ad1f = dp('as1f', [1, HF], F16), dp('ad1f', [1, HF], F16)
    as2f, ad2f = dp('as2f', [1, HF], F16), dp('ad2f', [1, HF], F16)
    wgcn = dp('wgcn_s', [128, F], F16)
    bgcn_col = dp('bgcn_col', [128, 1], F32)
    bgcn_row = dp('bgcn_row', [1, F], F32)
    bg2row = dp('bg2row', [1, HF], F32)
    wfg1p = dp('wfg1p', [14, 128, 128], F16)
    bfg1 = dp('bfg1', [1, 128], F32)
    wfg2p = dp('wfg2p', [2, 128, 128], F16)
    bfg2 = dp('bfg2', [1, 128], F32)
    wxtp = dp('wxtp', [5, 128, 256], F16)
    bxt = dp('bxt', [1, 256], F32)
    w1p = dp('w1p', [4, 128, 512], F16)
    b1 = dp('b1', [1, 512], F32)
    w2p = dp('w2p', [4, 128, 256], F16)
    b2 = dp('b2', [1, 256], F32)
    wop = dp('wop', [2, 128, 1], F16)
    bo_rep = dp('bo_rep', [GPC, 1], F32)
    wcol_d = dp('w_col', [32, 1], F16)
    wsel_d = dp('w_sel', [32, GPC, GPC], F16)

    out_d = nc.declare_dram_parameter('out', [GPC, 1], F32, isOutput=True)

    # internal DRAM
    CROW = 1024
    comb_shard = nc.dram_tensor('comb_shard', [NPC, CROW], F16)
    comb_full = nc.dram_tensor('comb_full', [8 * NPC, CROW], F16, addr_space="Shared")
    x1f_dram = nc.dram_tensor('x1f_dram', [NPC, TROW], F16)
    x2f_dram = nc.dram_tensor('x2f_dram', [NPC, XROW], F16)

    RG = [list(range(8))]

    with TileContext(nc) as tc:
        nc.gpsimd.load_library(library_config.mlp)

        with tc.tile_pool(name="persist", bufs=1) as pp:
            # ---------------- persistent tiles + loads
            w1aug_s = pp.tile([128, WAUG1], F16, tag="w1aug")
            nc.sync.dma_start(out=w1aug_s[:], in_=wg1cat[:])
            w2aug_s = pp.tile([128, 7, W2AUG], F16, tag="w2aug")
            for k in range(7):
                nc.sync.dma_start(out=w2aug_s[:, k, :], in_=w2ch[k])
            bg1_s = pp.tile([128, 7, 1], F16, tag="bg1")
            for k in range(7):
                nc.sync.dma_start(out=bg1_s[:, k, :], in_=bg1ch[k])
            a_s = pp.tile([128, 4, HF], F16, tag="aflat")
            for i, t in enumerate([as1f, ad1f, as2f, ad2f]):
                nc.sync.dma_start(out=a_s[:, i, :], in_=t[:].to_broadcast([128, HF]))
            xT_s = pp.tile([128, NPC], F16, tag="xT")
            nc.sync.dma_start(out=xT_s[:], in_=xT_loc[:])
            ixx_s = pp.tile([128, ECAP // 16], I16, tag="ixx")
            nc.sync.dma_start(out=ixx_s[:], in_=ix_x[:])
            ixt2_s = pp.tile([128, ECAP // 16], I16, tag="ixt2")
            nc.sync.dma_start(out=ixt2_s[:], in_=ix_t2[:])
            ixp_s = pp.tile([128, GPC * PW // 16], I16, tag="ixp")
            nc.sync.dma_start(out=ixp_s[:], in_=ix_pool[:])
            wgcn_s = pp.tile([128, F], F16, tag="wgcn")
            nc.sync.dma_start(out=wgcn_s[:], in_=wgcn[:])
            bgcnc_s = pp.tile([128, 1], F32, tag="bgcnc")
            nc.sync.dma_start(out=bgcnc_s[:], in_=bgcn_col[:])
            bgcnr_s = pp.tile([128, F], F32, tag="bgcnr")
            nc.sync.dma_start(out=bgcnr_s[:], in_=bgcn_row[:].to_broadcast([128, F]))
            bg2_s = pp.tile([128, HF], F32, tag="bg2")
            nc.sync.dma_start(out=bg2_s[:], in_=bg2row[:].to_broadcast([128, HF]))
            mmean_s = pp.tile([128, NBLK, GPC], F16, tag="mmean")
            for b in range(NBLK):
                nc.sync.dma_start(out=mmean_s[:, b, :], in_=mmean_d[b])
            wcol_s = pp.tile([32, 1], F16, tag="wcol")
            nc.sync.dma_start(out=wcol_s[:], in_=wcol_d[:])
            wsel_s = pp.tile([32, GPC, GPC], F16, tag="wsel")
            nc.sync.dma_start(out=wsel_s[:], in_=wsel_d[:])
            bconv_s = pp.tile([GPC, 1], F32, tag="bconv")
            nc.sync.dma_start(out=bconv_s[:], in_=bconv_rep[:])
            wfg1_s = pp.tile([128, 14, 128], F16, tag="wfg1")
            for i in range(14):
                nc.sync.dma_start(out=wfg1_s[:, i, :], in_=wfg1p[i])
            wfg2_s = pp.tile([128, 2, 128], F16, tag="wfg2")
            for i in range(2):
                nc.sync.dma_start(out=wfg2_s[:, i, :], in_=wfg2p[i])
            wxt_s = pp.tile([128, 5, 256], F16, tag="wxt")
            for i in range(5):
                nc.sync.dma_start(out=wxt_s[:, i, :], in_=wxtp[i])
            w1_s = pp.tile([128, 4, 512], F16, tag="w1")
            for i in range(4):
                nc.sync.dma_start(out=w1_s[:, i, :], in_=w1p[i])
            w2_s = pp.tile([128, 4, 256], F16, tag="w2")
            for i in range(4):
                nc.sync.dma_start(out=w2_s[:, i, :], in_=w2p[i])
            wo_s = pp.tile([128, 2, 1], F16, tag="wo")
            for i in range(2):
                nc.sync.dma_start(out=wo_s[:, i, :], in_=wop[i])
            bias_s = {}
            for nm, t, w in [('bfg1', bfg1, 128), ('bfg2', bfg2, 128),
                             ('bxt', bxt, 256), ('b1', b1, 512), ('b2', b2, 256)]:
                bias_s[nm] = pp.tile([GPC, w], F32, tag="bias_" + nm, name="bias_" + nm)
                nc.sync.dma_start(out=bias_s[nm][:], in_=t[:].to_broadcast([GPC, w]))
            bo_s = pp.tile([GPC, 1], F32, tag="bo")
            nc.sync.dma_start(out=bo_s[:], in_=bo_rep[:])

            ident_s = pp.tile([128, 128], F16, tag="ident")
            make_identity(nc, ident_s[:])
            ones_s = pp.tile([1, 128], F16, tag="ones")
            nc.vector.memset(ones_s[:], 1.0)

            # work state
            bd1_s = pp.tile([128, H], F16, tag="bd1")
            adst1_s = pp.tile([128, NBLK, H], F16, tag="adst1")
            adst2_s = pp.tile([128, NBLK, H], F16, tag="adst2")
            x1loc_s = pp.tile([128, NBLK, HF], F16, tag="x1loc")
            agg1_s = pp.tile([128, NBLK, F], F16, tag="agg1")
            c2_s = pp.tile([1, W2AUG], F16, tag="c2")
            exA = pp.tile([128, H + 1], F16, tag="exA")
            exB = pp.tile([128, H + 1], F16, tag="exB")
            nc.vector.memset(exA[:], 1.0)
            nc.vector.memset(exB[:], 1.0)
            t2stage = pp.tile([128, TROW], F16, tag="t2stage")
            nc.vector.memset(t2stage[:], 0.0)
            xstage = pp.tile([128, XROW], F16, tag="xstage")
            nc.vector.memset(xstage[:], 0.0)

            # ---------------- B matrices (device)
            with tc.tile_pool(name="bprep", bufs=2) as bp, \
                 tc.tile_pool(name="bprep_ps", bufs=2, space="PSUM") as bps:
                for (src_w, col0) in [(0, HF), (1, HF + H)]:   # as2 -> B_s2, ad2 -> B_d2
                    pass
                # B_s1 / B_d1 from Wg1 (rows of w1aug_s)
                for i, dst in enumerate(['s', 'd']):
                    tmp = bp.tile([128, HF], F32, tag="btmp")
                    nc.vector.tensor_tensor(
                        out=tmp[:], in0=w1aug_s[:, 0:HF],
                        in1=a_s[:, i, :], op=ALU.mult)
                    red = bp.tile([128, H], F32, tag="bred")
                    nc.vector.tensor_reduce(
                        out=red[:], in_=tmp[:].rearrange("p (h f) -> p h f", h=H),
                        op=ALU.add, axis=AX)
                    if i == 0:
                        nc.vector.tensor_copy(out=w1aug_s[:, HF + F:WAUG1], in_=red[:])
                    else:
                        nc.vector.tensor_copy(out=bd1_s[:], in_=red[:])
                # B_s2 / B_d2 per chunk of Wg2
                for k in range(7):
                    for i, col0 in [(2, HF), (3, HF + H)]:
                        tmp = bp.tile([128, HF], F32, tag="btmp")
                        nc.vector.tensor_tensor(
                            out=tmp[:], in0=w2aug_s[:, k, 0:HF],
                            in1=a_s[:, i, :], op=ALU.mult)
                        red = bp.tile([128, H], F32, tag="bred")
                        nc.vector.tensor_reduce(
                            out=red[:], in_=tmp[:].rearrange("p (h f) -> p h f", h=H),
                            op=ALU.add, axis=AX)
                        nc.vector.tensor_copy(out=w2aug_s[:, k, col0:col0 + H], in_=red[:])
                # c2 = bg1 @ W2aug
                ps_c2 = bps.tile([1, W2AUG], F32, space="PSUM", tag="psc2")
                for k in range(7):
                    nc.tensor.matmul(out=ps_c2[:, 0:512], lhsT=bg1_s[:, k, :],
                                     rhs=w2aug_s[:, k, 0:512], start=(k == 0), stop=(k == 6))
                    nc.tensor.matmul(out=ps_c2[:, 512:W2AUG], lhsT=bg1_s[:, k, :],
                                     rhs=w2aug_s[:, k, 512:W2AUG], start=(k == 0), stop=(k == 6))
                nc.vector.tensor_copy(out=c2_s[:], in_=ps_c2[:])
                # adst1 per block
                for b in range(NBLK):
                    ps_a = bps.tile([128, H], F32, space="PSUM", tag="psa")
                    nc.tensor.matmul(out=ps_a[:], lhsT=xT_s[:, 128 * b:128 * (b + 1)],
                                     rhs=bd1_s[:], start=True, stop=True)
                    nc.vector.tensor_copy(out=adst1_s[:, b, :], in_=ps_a[:])

            # ---------------- phase 1: GAT1 + GCN1 edge loop
            with tc.tile_pool(name="p1", bufs=3) as p1, \
                 tc.tile_pool(name="p1g", bufs=2) as p1g, \
                 tc.tile_pool(name="p1s", bufs=2, space="PSUM") as p1s, \
                 tc.tile_pool(name="p1acc", bufs=1, space="PSUM") as p1acc:
                for b in range(NBLK):
                    xgt = p1g.tile([128, 1, EPB], F16, tag="xgt")
                    nc.gpsimd.dma_gather(
                        out_ap=xgt[:], in_ap=x16[:],
                        idxs_ap=ixx_s[:, b * (EPB // 16):(b + 1) * (EPB // 16)],
                        num_idxs=EPB, num_idxs_reg=EPB, elem_size=XROW, transpose=True,
                        single_packet=False)
                    s01t_b = p1g.tile([128, EPB], F16, tag="s01tb")
                    nc.sync.dma_start(out=s01t_b[:], in_=s01t_d[b])
                    scomb_b = p1g.tile([128, TPB * 256], F16, tag="scombb")
                    nc.sync.dma_start(out=scomb_b[:], in_=scomb_d[b])
                    ps_out = p1acc.tile([128, HF], F32, space="PSUM", tag="psout", name="psout")[:]
                    ps_s = p1acc.tile([128, H], F32, space="PSUM", tag="pss", name="pss")[:]
                    ps_gcn = p1acc.tile([128, F], F32, space="PSUM", tag="psgcn", name="psgcn")[:]
                    for k in range(TPB):
                        s01_t = scomb_b[:, k * 256:k * 256 + 128]
                        snrm_t = scomb_b[:, k * 256 + 128:k * 256 + 256]
                        lhs = xgt[:, 0, 128 * k:128 * (k + 1)]
                        ps1 = p1s.tile([128, WAUG1], F32, space="PSUM", tag="ps1")
                        nc.tensor.matmul(out=ps1[:, 0:512], lhsT=lhs,
                                         rhs=w1aug_s[:, 0:512], start=True, stop=True)
                        nc.tensor.matmul(out=ps1[:, 512:WAUG1], lhsT=lhs,
                                         rhs=w1aug_s[:, 512:WAUG1], start=True, stop=False)
                        nc.tensor.matmul(out=ps1[:, HF + F:WAUG1],
                                         lhsT=s01t_b[:, 128 * k:128 * (k + 1)],
                                         rhs=adst1_s[:, b, :], start=False, stop=True)
                        ex = exA if k % 2 == 0 else exB
                        lr02 = p1.tile([128, H], F32, tag="lr02")
                        nc.scalar.activation(out=lr02[:], in_=ps1[:, HF + F:WAUG1],
                                             func=AF.Copy, scale=0.2)
                        lr = p1.tile([128, H], F32, tag="lr")
                        nc.vector.tensor_tensor(out=lr[:], in0=ps1[:, HF + F:WAUG1],
                                                in1=lr02[:], op=ALU.max)
                        nc.scalar.activation(out=ex[:, 0:H], in_=lr[:], func=AF.Exp)
                        exv = p1.tile([128, HF + F], F16, tag="exv")
                        nc.vector.tensor_tensor(
                            out=exv[:].rearrange("p (h f) -> p h f", h=H + 1),
                            in0=ps1[:, 0:HF + F].rearrange("p (h f) -> p h f", h=H + 1),
                            in1=ex[:, :, None].to_broadcast([128, H + 1, F]),
                            op=ALU.mult)
                        nc.tensor.matmul(out=ps_s, lhsT=s01_t, rhs=ex[:, 0:H],
                                         start=(k == 0), stop=(k == TPB - 1))
                        nc.tensor.matmul(out=ps_out[:, 0:512], lhsT=s01_t,
                                         rhs=exv[:, 0:512], start=(k == 0), stop=(k == TPB - 1))
                        nc.tensor.matmul(out=ps_out[:, 512:HF], lhsT=s01_t,
                                         rhs=exv[:, 512:HF], start=(k == 0), stop=(k == TPB - 1))
                        nc.tensor.matmul(out=ps_gcn, lhsT=snrm_t,
                                         rhs=exv[:, HF:HF + F], start=(k == 0), stop=(k == TPB - 1))
                    rec = p1.tile([128, H], F32, tag="rec")
                    nc.vector.reciprocal(out=rec[:], in_=ps_s)
                    nc.vector.tensor_tensor(
                        out=x1loc_s[:, b, :].rearrange("p (h f) -> p h f", h=H),
                        in0=ps_out[:].rearrange("p (h f) -> p h f", h=H),
                        in1=rec[:, :, None].to_broadcast([128, H, F]),
                        op=ALU.mult)
                    nc.vector.tensor_copy(out=agg1_s[:, b, :], in_=ps_gcn)

            # ---------------- phase 2: x2 table, T2 table, collectives, conv
            with tc.tile_pool(name="p2", bufs=2) as p2:
              with tc.tile_pool(name="p2sa", bufs=2, space="PSUM") as p2s, \
                   tc.tile_pool(name="p2ta", bufs=2, space="PSUM") as p2t:
                # x2 table shard + allgather (early, small)
                for b in range(NBLK):
                    psT = p2t.tile([128, 128], F16, space="PSUM", tag="psT")
                    nc.tensor.transpose(out=psT[:F, :], in_=agg1_s[:, b, :],
                                        identity=ident_s[:])
                    x2lt = p2.tile([128, 128], F16, tag="x2lt")
                    nc.vector.tensor_scalar(out=x2lt[:F, :], in0=psT[:F, :],
                                            scalar1=bgcnc_s[:F, :], scalar2=None,
                                            op0=ALU.add)
                    ps_x2 = p2s.tile([128, F], F32, space="PSUM", tag="psx2")
                    nc.tensor.matmul(out=ps_x2[:], lhsT=x2lt[:F, :], rhs=wgcn_s[:F, :],
                                     start=True, stop=True)
                    nc.vector.tensor_copy(out=xstage[:, 0:F], in_=ps_x2[:])
                    nc.sync.dma_start(out=comb_shard[128 * b:128 * (b + 1), 0:XROW],
                                      in_=xstage[:])
              with tc.tile_pool(name="p2sb", bufs=2, space="PSUM") as p2s, \
                   tc.tile_pool(name="p2tb", bufs=2, space="PSUM") as p2t:
                # x1loc transposes -> x1t_s
                x1t_s = p2.tile([128, 7, NPC], F16, tag="x1t", bufs=1)
                nc.vector.memset(x1t_s[:], 0.0)
                for b in range(NBLK):
                    for fb in range(7):
                        c0, c1 = 128 * fb, min(128 * (fb + 1), HF)
                        psT = p2t.tile([128, 128], F16, space="PSUM", tag="psT")
                        nc.tensor.transpose(out=psT[:c1 - c0, :],
                                            in_=x1loc_s[:, b, c0:c1],
                                            identity=ident_s[:])
                        nc.vector.tensor_copy(
                            out=x1t_s[0:c1 - c0, fb, 128 * b:128 * (b + 1)],
                            in_=psT[:c1 - c0, :])
                # T2 build
                for b in range(NBLK):
                    ps_t2 = p2s.tile([128, W2AUG], F32, space="PSUM", tag="pst2")
                    for k in range(7):
                        nc.tensor.matmul(out=ps_t2[:, 0:512],
                                         lhsT=x1t_s[:, k, 128 * b:128 * (b + 1)],
                                         rhs=w2aug_s[:, k, 0:512], start=(k == 0), stop=False)
                        nc.tensor.matmul(out=ps_t2[:, 512:W2AUG],
                                         lhsT=x1t_s[:, k, 128 * b:128 * (b + 1)],
                                         rhs=w2aug_s[:, k, 512:W2AUG], start=(k == 0), stop=False)
                    nc.tensor.matmul(out=ps_t2[:, 0:512], lhsT=ones_s[:],
                                     rhs=c2_s[:, 0:512], start=False, stop=True)
                    nc.tensor.matmul(out=ps_t2[:, 512:W2AUG], lhsT=ones_s[:],
                                     rhs=c2_s[:, 512:W2AUG], start=False, stop=True)
                    nc.vector.tensor_copy(out=t2stage[:, 0:T2W], in_=ps_t2[:, 0:T2W])
                    nc.vector.tensor_copy(out=adst2_s[:, b, :], in_=ps_t2[:, T2W:W2AUG])
                    nc.sync.dma_start(out=comb_shard[128 * b:128 * (b + 1), XROW:CROW],
                                      in_=t2stage[:])
                nc.gpsimd.collective_compute(
                    "AllGather", ALU.bypass, replica_groups=RG,
                    ins=[comb_shard[:]], outs=[comb_full[:]])

              with tc.tile_pool(name="p2sc", bufs=1, space="PSUM") as p2s, \
                   tc.tile_pool(name="p2tc", bufs=2, space="PSUM") as p2t:
                # conv branch (runs during the collectives)
                twin_s = p2.tile([32, GPC, 608], F16, tag="twin", bufs=1)
                nc.sync.dma_start(out=twin_s[:], in_=twin_d[:])
                ps_ya = p2s.tile([GPC, 512], F32, space="PSUM", tag="psya")
                ps_yb = p2s.tile([GPC, 96], F32, space="PSUM", tag="psyb")
                for g in range(GPC):
                    nc.tensor.matmul(out=ps_ya[:], lhsT=wsel_s[:, g, :],
                                     rhs=twin_s[:, g, 0:512], start=(g == 0), stop=(g == GPC - 1))
                    nc.tensor.matmul(out=ps_yb[:], lhsT=wsel_s[:, g, :],
                                     rhs=twin_s[:, g, 512:608], start=(g == 0), stop=(g == GPC - 1))
                y_s = p2.tile([GPC, 608], F16, tag="ys")
                nc.vector.tensor_scalar(out=y_s[:, 0:512], in0=ps_ya[:],
                                        scalar1=bconv_s[:], scalar2=0.0,
                                        op0=ALU.add, op1=ALU.max)
                nc.vector.tensor_scalar(out=y_s[:, 512:608], in0=ps_yb[:],
                                        scalar1=bconv_s[:], scalar2=0.0,
                                        op0=ALU.add, op1=ALU.max)
                yt_s = pp.tile([128, 5, GPC], F16, tag="yt")
                nc.vector.memset(yt_s[:], 0.0)
                for i in range(5):
                    c0, c1 = 128 * i, min(128 * (i + 1), 608)
                    psT = p2t.tile([128, 128], F16, space="PSUM", tag="psT")
                    nc.tensor.transpose(out=psT[:c1 - c0, :GPC], in_=y_s[:, c0:c1],
                                        identity=ident_s[:GPC, :GPC])
                    nc.vector.tensor_copy(out=yt_s[0:c1 - c0, i, :], in_=psT[:c1 - c0, :GPC])
                ps_xt = p2s.tile([GPC, 256], F32, space="PSUM", tag="psxt")
                for i in range(5):
                    nc.tensor.matmul(out=ps_xt[:], lhsT=yt_s[:, i, :], rhs=wxt_s[:, i, :],
                                     start=(i == 0), stop=(i == 4))
                xt_s = p2.tile([GPC, 256], F16, tag="xts")
                nc.vector.tensor_tensor(out=xt_s[:], in0=ps_xt[:],
                                        in1=bias_s['bxt'][:],
                                        op=ALU.add)
                xtT_s = pp.tile([128, 2, GPC], F16, tag="xtT")
                for i in range(2):
                    psT = p2t.tile([128, 128], F16, space="PSUM", tag="psT")
                    nc.tensor.transpose(out=psT[:, :GPC], in_=xt_s[:, 128 * i:128 * (i + 1)],
                                        identity=ident_s[:GPC, :GPC])
                    nc.vector.tensor_copy(out=xtT_s[:, i, :], in_=psT[:, :GPC])

            # ---------------- phase 3: GAT2 + GCN2 edge loop
            with tc.tile_pool(name="p3", bufs=3) as p3, \
                 tc.tile_pool(name="p3g", bufs=2) as p3g, \
                 tc.tile_pool(name="p3s", bufs=2, space="PSUM") as p3s, \
                 tc.tile_pool(name="p3acc", bufs=1, space="PSUM") as p3acc:
                for b in range(NBLK):
                    v2g = p3g.tile([128, TPB, TROW], F16, tag="v2g")
                    nc.gpsimd.dma_gather(
                        out_ap=v2g[:], in_ap=comb_full[:, XROW:CROW],
                        idxs_ap=ixt2_s[:, b * (EPB // 16):(b + 1) * (EPB // 16)],
                        num_idxs=EPB, num_idxs_reg=EPB, elem_size=TROW, elem_step=CROW,
                        single_packet=False)
                    vxg = p3g.tile([128, TPB, XROW], F16, tag="vxg")
                    nc.gpsimd.dma_gather(
                        out_ap=vxg[:], in_ap=comb_full[:, 0:XROW],
                        idxs_ap=ixt2_s[:, b * (EPB // 16):(b + 1) * (EPB // 16)],
                        num_idxs=EPB, num_idxs_reg=EPB, elem_size=XROW, elem_step=CROW,
                        single_packet=False)
                    s01t_b = p3g.tile([128, EPB], F16, tag="s01tb3")
                    nc.sync.dma_start(out=s01t_b[:], in_=s01t_d[b])
                    scomb_b = p3g.tile([128, TPB * 256], F16, tag="scombb3")
                    nc.sync.dma_start(out=scomb_b[:], in_=scomb_d[b])
                    ps_out = p3acc.tile([128, HF], F32, space="PSUM", tag="psout3", name="psout3")[:]
                    ps_s = p3acc.tile([128, H], F32, space="PSUM", tag="pss3", name="pss3")[:]
                    ps_g2 = p3acc.tile([128, F], F32, space="PSUM", tag="psg2", name="psg2")[:]
                    for k in range(TPB):
                        s01_t = scomb_b[:, k * 256:k * 256 + 128]
                        snrm_t = scomb_b[:, k * 256 + 128:k * 256 + 256]
                        ps_l = p3s.tile([128, H], F32, space="PSUM", tag="psl")
                        nc.tensor.matmul(out=ps_l[:], lhsT=s01t_b[:, 128 * k:128 * (k + 1)],
                                         rhs=adst2_s[:, b, :], start=True, stop=False)
                        nc.tensor.matmul(out=ps_l[:], lhsT=ident_s[:],
                                         rhs=v2g[:, k, HF:T2W], start=False, stop=True)
                        ex = exA if k % 2 == 0 else exB
                        lr02 = p3.tile([128, H], F32, tag="lr023")
                        nc.scalar.activation(out=lr02[:], in_=ps_l[:], func=AF.Copy, scale=0.2)
                        lr = p3.tile([128, H], F32, tag="lr3")
                        nc.vector.tensor_tensor(out=lr[:], in0=ps_l[:], in1=lr02[:], op=ALU.max)
                        nc.scalar.activation(out=ex[:, 0:H], in_=lr[:], func=AF.Exp)
                        exv = p3.tile([128, HF], F16, tag="exv3")
                        nc.vector.tensor_tensor(
                            out=exv[:].rearrange("p (h f) -> p h f", h=H),
                            in0=v2g[:, k, 0:HF].rearrange("p (h f) -> p h f", h=H),
                            in1=ex[:, 0:H, None].to_broadcast([128, H, F]),
                            op=ALU.mult)
                        nc.tensor.matmul(out=ps_s, lhsT=s01_t, rhs=ex[:, 0:H],
                                         start=(k == 0), stop=(k == TPB - 1))
                        nc.tensor.matmul(out=ps_out[:, 0:512], lhsT=s01_t,
                                         rhs=exv[:, 0:512], start=(k == 0), stop=(k == TPB - 1))
                        nc.tensor.matmul(out=ps_out[:, 512:HF], lhsT=s01_t,
                                         rhs=exv[:, 512:HF], start=(k == 0), stop=(k == TPB - 1))
                        nc.tensor.matmul(out=ps_g2, lhsT=snrm_t,
                                         rhs=vxg[:, k, 0:F], start=(k == 0), stop=(k == TPB - 1))
                    rec = p3.tile([128, H], F32, tag="rec3")
                    nc.vector.reciprocal(out=rec[:], in_=ps_s)
                    u_s = p3.tile([128, HF], F16, tag="us")
                    nc.vector.tensor_tensor(
                        out=u_s[:].rearrange("p (h f) -> p h f", h=H),
                        in0=ps_out.rearrange("p (h f) -> p h f", h=H),
                        in1=rec[:, :, None].to_broadcast([128, H, F]),
                        op=ALU.mult)
                    v_s = p3.tile([128, HF], F16, tag="vs")
                    nc.vector.tensor_tensor(out=v_s[:], in0=u_s[:],
                                            in1=bg2_s[:],
                                            op=ALU.add)
                    nc.vector.tensor_scalar(out=t2stage[:, 0:HF], in0=v_s[:],
                                            scalar1=0.0, scalar2=None, op0=ALU.max)
                    nc.sync.dma_start(out=x1f_dram[128 * b:128 * (b + 1), :], in_=t2stage[:])
                    g2f = p3.tile([128, F], F32, tag="g2f")
                    nc.vector.tensor_tensor(out=g2f[:], in0=ps_g2,
                                            in1=bgcnr_s[:],
                                            op=ALU.add)
                    nc.vector.tensor_scalar(out=xstage[:, 0:F], in0=g2f[:],
                                            scalar1=0.0, scalar2=None, op0=ALU.max)
                    nc.sync.dma_start(out=x2f_dram[128 * b:128 * (b + 1), :], in_=xstage[:])

            # ---------------- phase 4: pooling + head
            with tc.tile_pool(name="p4", bufs=2) as p4:
              with tc.tile_pool(name="p4s", bufs=1, space="PSUM") as p4s:
                pass
                x1f_s = p4.tile([128, NBLK, HF], F16, tag="x1fp4", bufs=1)
                for b in range(NBLK):
                    nc.sync.dma_start(out=x1f_s[:, b, :],
                                      in_=x1f_dram[128 * b:128 * (b + 1), 0:HF])
                x2f_s = p4.tile([128, NBLK, F], F16, tag="x2fp4", bufs=1)
                for b in range(NBLK):
                    nc.sync.dma_start(out=x2f_s[:, b, :],
                                      in_=x2f_dram[128 * b:128 * (b + 1), 0:F])
                gmax1T = pp.tile([128, 7, GPC], F16, tag="gmax1T")
                gmax2T = pp.tile([128, 1, GPC], F16, tag="gmax2T")
                CH = GPC // 2
                for h in range(2):
                    slab = p4.tile([128, 7, CH * PW], F16, tag="slab")
                    nc.gpsimd.dma_gather(
                        out_ap=slab[:], in_ap=x1f_dram[:],
                        idxs_ap=ixp_s[:, h * (CH * PW // 16):(h + 1) * (CH * PW // 16)],
                        num_idxs=CH * PW, num_idxs_reg=CH * PW, elem_size=TROW,
                        transpose=True, single_packet=False)
                    for g in range(CH):
                        for j in range(7):
                            nc.vector.tensor_reduce(
                                out=gmax1T[:, j, h * CH + g:h * CH + g + 1],
                                in_=slab[:, j, g * PW:(g + 1) * PW],
                                op=ALU.max, axis=AX)
                    slab2 = p4.tile([128, 1, CH * PW], F16, tag="slab2")
                    nc.gpsimd.dma_gather(
                        out_ap=slab2[:], in_ap=x2f_dram[:],
                        idxs_ap=ixp_s[:, h * (CH * PW // 16):(h + 1) * (CH * PW // 16)],
                        num_idxs=CH * PW, num_idxs_reg=CH * PW, elem_size=XROW,
                        transpose=True, single_packet=False)
                    for g in range(CH):
                        nc.vector.tensor_reduce(
                            out=gmax2T[:, 0, h * CH + g:h * CH + g + 1],
                            in_=slab2[:, 0, g * PW:(g + 1) * PW],
                            op=ALU.max, axis=AX)
              # means via matmul, then transpose
              with tc.tile_pool(name="p4sm", bufs=1, space="PSUM") as p4s:
                ps_m1 = p4s.tile([GPC, HF], F32, space="PSUM", tag="psm1")
                ps_m2 = p4s.tile([GPC, F], F32, space="PSUM", tag="psm2")
                for b in range(NBLK):
                    nc.tensor.matmul(out=ps_m1[:, 0:512], lhsT=mmean_s[:, b, :],
                                     rhs=x1f_s[:, b, 0:512], start=(b == 0), stop=(b == NBLK - 1))
                    nc.tensor.matmul(out=ps_m1[:, 512:HF], lhsT=mmean_s[:, b, :],
                                     rhs=x1f_s[:, b, 512:HF], start=(b == 0), stop=(b == NBLK - 1))
                    nc.tensor.matmul(out=ps_m2[:], lhsT=mmean_s[:, b, :],
                                     rhs=x2f_s[:, b, :], start=(b == 0), stop=(b == NBLK - 1))
                mean1 = p4.tile([GPC, HF], F16, tag="mean1")
                nc.vector.tensor_copy(out=mean1[:], in_=ps_m1[:])
                mean2 = p4.tile([GPC, F], F16, tag="mean2")
                nc.vector.tensor_copy(out=mean2[:], in_=ps_m2[:])
              with tc.tile_pool(name="p4sh", bufs=1, space="PSUM") as p4s:
                gmean1T = pp.tile([128, 7, GPC], F16, tag="gmean1T")
                nc.vector.memset(gmean1T[:], 0.0)
                gmean2T = pp.tile([128, 1, GPC], F16, tag="gmean2T")
                nc.vector.memset(gmean2T[:], 0.0)
                for i in range(7):
                    c0, c1 = 128 * i, min(128 * (i + 1), HF)
                    psT = p4s.tile([128, 128], F16, space="PSUM", tag="psT4", bufs=2)
                    nc.tensor.transpose(out=psT[:c1 - c0, :GPC], in_=mean1[:, c0:c1],
                                        identity=ident_s[:GPC, :GPC])
                    nc.vector.tensor_copy(out=gmean1T[0:c1 - c0, i, :], in_=psT[:c1 - c0, :GPC])
                psT = p4s.tile([128, 128], F16, space="PSUM", tag="psT4", bufs=2)
                nc.tensor.transpose(out=psT[:F, :GPC], in_=mean2[:], identity=ident_s[:GPC, :GPC])
                nc.vector.tensor_copy(out=gmean2T[0:F, 0, :], in_=psT[:F, :GPC])

                def head_mm(ps, chunks, rhs_tile, nw):
                    n = len(chunks)
                    for i, ch in enumerate(chunks):
                        nc.tensor.matmul(out=ps[:], lhsT=ch, rhs=rhs_tile[:, i, :nw],
                                         start=(i == 0), stop=(i == n - 1))

                def bias_relu_T(ps, bias_ap, w, relu, nT, tagb):
                    zs = p4.tile([GPC, w], F16, tag="z" + tagb)
                    nc.vector.tensor_tensor(out=zs[:], in0=ps[:],
                                            in1=bias_ap, op=ALU.add)
                    if relu:
                        nc.vector.tensor_scalar(out=zs[:], in0=zs[:], scalar1=0.0,
                                                scalar2=None, op0=ALU.max)
                    zT = pp.tile([128, nT, GPC], F16, tag="zT" + tagb)
                    for i in range(nT):
                        psT2 = p4s.tile([128, 128], F16, space="PSUM", tag="psT4", bufs=2)
                        nc.tensor.transpose(out=psT2[:, :GPC], in_=zs[:, 128 * i:128 * (i + 1)],
                                            identity=ident_s[:GPC, :GPC])
                        nc.vector.tensor_copy(out=zT[:, i, :], in_=psT2[:, :GPC])
                    return zT

                ps_z1 = p4s.tile([GPC, 128], F32, space="PSUM", tag="psz1")
                head_mm(ps_z1, [gmax1T[:, j, :] for j in range(7)]
                        + [gmean1T[:, j, :] for j in range(7)], wfg1_s, 128)
                z1T = bias_relu_T(ps_z1, bias_s['bfg1'][:], 128, True, 1, "1")
                ps_z2 = p4s.tile([GPC, 128], F32, space="PSUM", tag="psz2")
                head_mm(ps_z2, [gmax2T[:, 0, :], gmean2T[:, 0, :]], wfg2_s, 128)
                z2T = bias_relu_T(ps_z2, bias_s['bfg2'][:], 128, True, 1, "2")
                ps_h1 = p4s.tile([GPC, 512], F32, space="PSUM", tag="psh1")
                head_mm(ps_h1, [z1T[:, 0, :], z2T[:, 0, :], xtT_s[:, 0, :], xtT_s[:, 1, :]],
                        w1_s, 512)
                h1T = bias_relu_T(ps_h1, bias_s['b1'][:], 512, True, 4, "h1")
                ps_h2 = p4s.tile([GPC, 256], F32, space="PSUM", tag="psh2")
                head_mm(ps_h2, [h1T[:, i, :] for i in range(4)], w2_s, 256)
                h2T = bias_relu_T(ps_h2, bias_s['b2'][:], 256, True, 2, "h2")
                ps_o = p4s.tile([GPC, 1], F32, space="PSUM", tag="pso")
                head_mm(ps_o, [h2T[:, i, :] for i in range(2)], wo_s, 1)
                o_s = p4.tile([GPC, 1], F32, tag="os")
                nc.vector.tensor_scalar(out=o_s[:], in0=ps_o[:], scalar1=bo_s[:],
                                        scalar2=None, op0=ALU.add)
                nc.sync.dma_start(out=out_d[:], in_=o_s[:])

    nc.compile()
    return nc


def build_in_maps(nc, shared, cores):
    declared = set()
    import concourse.mybir as _mb
    for alloc in nc.m.functions[0].allocations:
        if isinstance(alloc, _mb.MemoryLocationSet) and alloc.kind == "ExternalInput":
            declared.add(alloc.memorylocations[0].name)
    in_maps = []
    for c in range(8):
        m = dict(shared)
        m.update(cores[c])
        in_maps.append({k: np.ascontiguousarray(v) for k, v in m.items()
                        if k in declared})
    return in_maps


_CACHE = {}


def run_device(inputs):
    meta, shared, cores = prep(**inputs)
    key = (meta['NBLK'], meta['TPB'], meta['PW'])
    if key not in _CACHE:
        _CACHE[key] = build(meta)
    nc = _CACHE[key]
    in_maps = build_in_maps(nc, shared, cores)
    res = run_bass_kernel_spmd(nc, in_maps, core_ids=list(range(8)))
    out = np.concatenate([res.results[c]['out'] for c in range(8)], axis=0)
    return out.astype(np.float32)


def kernel(**inputs):
    return run_device(inputs)



# revision 8
# speedup vs baseline: 1.0730x; 1.0730x over previous
"""Trainium2 Bass kernel for nn_GAT_GCN (gnn_message_passing), 8 NeuronCores.

v2 strategy (from v1 baseline at 1075us):
 - Dst-node sharding, graph-aligned (16 graphs/core). Within a core, nodes
   are BIN-PACKED into 128-node blocks balancing per-block edge counts, so
   every block needs the same tile count (TPB~9 vs v1's 10) -> 153 tiles
   instead of 170.
 - One merged exchange row per node: [h2(780) | x2(78) | asrc2(10) | pad]
   = 896 f16 = 1792B. One AllGather (31.2MB) and ONE phase-3 gather per
   edge (v1 had two gathers + 1024-col rows).
 - Scatter matrices s01/s01t in fp8 (exact 0/1); GCN's norm is applied via
   an extra broadcast column in the exp multiply (ex_ext[:,10]=norm_e), so
   the separate snrm matrix is gone.
 - Per-edge scale multiply (exv) runs in DVE 2x mode: PSUM->SBUF fp16 copy
   on ACT for most tiles, direct-PSUM for the rest (engine balance).
 - Leakyrelu max on gpsimd; denominators folded into the single scatter
   matmul pair (rhs = [exv(858) | ex(10)]).
 - Mean-pool accumulated inline in phase 3 (PE matmul); max-pool gathers
   read one merged x1f|x2f table; reduces split DVE/gpsimd.
"""
import sys
sys.path.insert(0, '/opt/trn_rl_repo')
import numpy as np
import ml_dtypes

N, E, G, F, H = 16384, 131072, 128, 78, 10
NCORE, GPC = 8, 16
HF = H * F                  # 780
WAUG1 = HF + F + H          # 868 = h1(780) | hgcn(78) | asrc(10)
W2AUG = HF + 2 * H          # 800 = h2 | asrc2 | adst2
CROW = 896                  # comb row: h2(780) | x2(78) | asrc2(10) | pad
XROW = 128                  # x table row, fp16 (256B)
EXW = WAUG1                 # 868 = exv(858) | ex(10) scatter rhs width


def _wrap16(v):
    v = np.asarray(v, np.int16)
    assert len(v) % 16 == 0
    m = v.reshape(-1, 16).T
    return np.tile(m, (8, 1)).copy()


def _f16(a):
    return np.ascontiguousarray(np.asarray(a, np.float32)).astype(np.float16)


def _f8(a):
    return np.ascontiguousarray(np.asarray(a, np.float32)).astype(ml_dtypes.float8_e4m3)


def prep(x, edge_index, batch, target, Wg1, as1, ad1, bg1, Wg2, as2, ad2, bg2,
         Wgcn, bgcn, Wfg1, bfg1, Wfg2, bfg2, wconv, bconv, Wxt, bxt,
         W1, b1, W2, b2, Wo, bo):
    x = np.asarray(x, np.float32)
    ei = np.asarray(edge_index, np.int64)
    batch = np.asarray(batch, np.int64)
    target = np.asarray(target, np.float32)

    loops = np.arange(N, dtype=np.int64)
    src = np.concatenate([ei[0], loops])
    dst = np.concatenate([ei[1], loops])

    counts = np.bincount(batch, minlength=G)
    node_off = np.concatenate([[0], np.cumsum(counts)])
    n_lo = node_off[np.arange(NCORE) * GPC]
    n_hi = node_off[(np.arange(NCORE) + 1) * GPC]

    deg = np.bincount(dst, minlength=N).astype(np.float64)
    dinv = 1.0 / np.sqrt(deg)
    norm = (dinv[src] * dinv[dst]).astype(np.float32)

    Lmax = int((n_hi - n_lo).max())
    NBLK = (Lmax + 127) // 128
    NPC = NBLK * 128
    assert NCORE * NPC < 32768

    # ---- per-core bin packing of nodes into NBLK blocks of <=128 nodes,
    # balancing per-block edge (degree) sums.
    blk_of = np.zeros(N, np.int64)     # block index of node (within its core)
    slot_of = np.zeros(N, np.int64)    # slot within block
    tiles_cb = np.zeros((NCORE, NBLK), np.int64)
    for c in range(NCORE):
        ids = np.arange(n_lo[c], n_hi[c])
        degs = deg[ids]
        order = np.argsort(-degs, kind='stable')
        bins_e = np.zeros(NBLK)
        bins_n = np.zeros(NBLK, np.int64)
        for i in order:
            cand = np.where(bins_n < 128, bins_e, np.inf)
            bsel = int(np.argmin(cand))
            nid = ids[i]
            blk_of[nid] = bsel
            slot_of[nid] = bins_n[bsel]
            bins_e[bsel] += degs[i]
            bins_n[bsel] += 1
        tiles_cb[c] = (bins_e.astype(np.int64) + 127) // 128
    TPBb = tiles_cb.max(axis=0)        # per-block tile count (same all cores)
    toff = np.concatenate([[0], np.cumsum(TPBb)])
    ET = int(toff[-1])
    ECAP = ET * 128
    node_owner = np.searchsorted(n_hi - 1, np.arange(N), side='left')
    node_owner = np.minimum(node_owner, NCORE - 1)
    local_id = blk_of * 128 + slot_of                    # 0..NPC-1
    pad_gid = node_owner * NPC + local_id

    # edges sorted by (core, block)
    ecore = node_owner[dst]
    eblk = blk_of[dst]
    ekey = ecore * NBLK + eblk
    order = np.argsort(ekey, kind='stable')
    srcs, dsts = src[order], dst[order]
    norms = norm[order]
    ekey_s = ekey[order]

    PW = int(np.ceil(counts.max() / 16) * 16)

    cores = []
    for c in range(NCORE):
        esrc = np.zeros(ECAP, np.int64)
        s01 = np.zeros((ET, 128, 128), ml_dtypes.float8_e4m3)
        s01t = np.zeros((ET, 128, 128), ml_dtypes.float8_e4m3)
        enrm = np.zeros((ET, 128), np.float16)    # [tile, edge-slot] norm
        for b in range(NBLK):
            lo = np.searchsorted(ekey_s, c * NBLK + b)
            hi = np.searchsorted(ekey_s, c * NBLK + b, side='right')
            ne = hi - lo
            t0 = toff[b]
            if ne > 0:
                j = np.arange(ne)
                t_loc = j // 128
                e_loc = j % 128
                ld = slot_of[dsts[lo:hi]]
                gslot = (t0 + t_loc) * 128 + e_loc
                esrc[gslot] = srcs[lo:hi]
                s01[t0 + t_loc, e_loc, ld] = 1.0
                s01t[t0 + t_loc, ld, e_loc] = 1.0
                enrm[t0 + t_loc, e_loc] = norms[lo:hi].astype(np.float16)
            # pad dst slots (no nodes) get a fake denominator entry
            nnode = int(((node_owner == c) & (blk_of == b)).sum()) if False else None
        # count nodes per (c, b) to set fake denominators on empty slots
        nb = np.zeros(NBLK, np.int64)
        sel = np.arange(n_lo[c], n_hi[c])
        for b in range(NBLK):
            nb[b] = int((blk_of[sel] == b).sum())
        for b in range(NBLK):
            if nb[b] < 128:
                s01[toff[b], 0, nb[b]:] = 1.0

        # pooling indices (local node ids into xf table)
        pool_idx = np.zeros(GPC * PW, np.int64)
        for g in range(GPC):
            gg = c * GPC + g
            ids = np.arange(node_off[gg], node_off[gg + 1])
            lid = local_id[ids]
            cnt = len(ids)
            pool_idx[g * PW:g * PW + cnt] = lid
            pool_idx[g * PW + cnt:(g + 1) * PW] = lid[0]
        mmean = np.zeros((NBLK, 128, GPC), np.float16)
        for g in range(GPC):
            gg = c * GPC + g
            ids = np.arange(node_off[gg], node_off[gg + 1])
            mmean[blk_of[ids], slot_of[ids], g] = np.float16(1.0 / len(ids))

        t_win = np.zeros((32, GPC, 608), np.float16)
        tg = target[c * GPC:(c + 1) * GPC, 0, :]
        for k in range(32):
            t_win[k, :, :594] = tg[:, k:k + 594].astype(np.float16)

        xT = np.zeros((128, NPC), np.float16)
        ids = np.arange(n_lo[c], n_hi[c])
        xT[:F, local_id[ids]] = x[ids, :].T.astype(np.float16)

        # combined fp8 s-matrix stream: [partition, tile*(s01|s01t)]
        s8 = np.zeros((128, ET, 256), ml_dtypes.float8_e4m3)
        s8[:, :, 0:128] = s01.transpose(1, 0, 2)
        s8[:, :, 128:256] = s01t.transpose(1, 0, 2)
        s8 = s8.reshape(128, ET * 256)
        cores.append(dict(
            ix_x=_wrap16(esrc),
            ix_t2=_wrap16(pad_gid[esrc]),
            ix_pool=_wrap16(pool_idx),
            s8=s8, enrm=np.ascontiguousarray(enrm.T),
            mmean=mmean, t_win=t_win, xT_loc=xT,
            bconv_rep=np.full((GPC, 1), float(bconv[0]), np.float32),
        ))

    x16 = np.zeros((N, XROW), np.float16)
    x16[:, :F] = x.astype(np.float16)

    Wg1cat = np.zeros((128, WAUG1), np.float16)
    Wg1cat[:F, :HF] = _f16(Wg1)
    Wg1cat[:F, HF:HF + F] = _f16(Wgcn)
    W2chunks = np.zeros((7, 128, W2AUG), np.float16)
    for k in range(7):
        r0, r1 = 128 * k, min(128 * (k + 1), HF)
        W2chunks[k, :r1 - r0, :HF] = _f16(Wg2[r0:r1, :])
    bg1ch = np.zeros((7, 128, 1), np.float16)
    bg1f = _f16(bg1).reshape(-1)
    for k in range(7):
        r0, r1 = 128 * k, min(128 * (k + 1), HF)
        bg1ch[k, :r1 - r0, 0] = bg1f[r0:r1]

    def pack_rows(Wm, splits, ncol):
        out = np.zeros((len(splits), 128, ncol), np.float16)
        for i, (r0, r1) in enumerate(splits):
            out[i, :r1 - r0, :] = _f16(Wm[r0:r1, :])
        return out

    sp7 = [(128 * i, min(128 * (i + 1), HF)) for i in range(7)]
    wfg1p = np.concatenate([pack_rows(Wfg1[:HF], sp7, 128),
                            pack_rows(Wfg1[HF:], sp7, 128)], axis=0)
    wfg2p = pack_rows(Wfg2, [(0, F), (F, 2 * F)], 128)
    wxtp = pack_rows(Wxt, [(128 * i, min(128 * (i + 1), 594)) for i in range(5)], 256)
    w1p = pack_rows(W1, [(128 * i, 128 * (i + 1)) for i in range(4)], 512)
    w2p = pack_rows(W2, [(128 * i, 128 * (i + 1)) for i in range(4)], 256)
    wop = pack_rows(Wo, [(0, 128), (128, 256)], 1)

    wgcn_s = np.zeros((128, F), np.float16)
    wgcn_s[:F] = _f16(Wgcn)
    bgcn_col = np.zeros((128, 1), np.float32)
    bgcn_col[:F, 0] = np.asarray(bgcn, np.float32)

    shared = dict(
        x16=x16, Wg1cat=Wg1cat, W2chunks=W2chunks, bg1ch=bg1ch,
        as1f=_f16(as1).reshape(1, HF), ad1f=_f16(ad1).reshape(1, HF),
        as2f=_f16(as2).reshape(1, HF), ad2f=_f16(ad2).reshape(1, HF),
        wgcn_s=wgcn_s, bgcn_col=bgcn_col,
        bgcn_row=np.asarray(bgcn, np.float32).reshape(1, F),
        bg2row=_f16(bg2).reshape(1, HF),
        wfg1p=wfg1p, bfg1=np.asarray(bfg1, np.float32).reshape(1, 128),
        wfg2p=wfg2p, bfg2=np.asarray(bfg2, np.float32).reshape(1, 128),
        wxtp=wxtp, bxt=np.asarray(bxt, np.float32).reshape(1, 256),
        w1p=w1p, b1=np.asarray(b1, np.float32).reshape(1, 512),
        w2p=w2p, b2=np.asarray(b2, np.float32).reshape(1, 256),
        wop=wop, bo_rep=np.full((GPC, 1), float(np.asarray(bo).reshape(-1)[0]), np.float32),
        w_col=np.zeros((32, 1), np.float16),
        w_sel=np.zeros((32, GPC, GPC), np.float16),
    )
    shared['w_col'][:, 0] = _f16(np.asarray(wconv).reshape(-1))
    for g in range(GPC):
        shared['w_sel'][:, g, g] = shared['w_col'][:, 0]

    meta = dict(NBLK=NBLK, NPC=NPC, TPBb=tuple(int(t) for t in TPBb),
                ET=ET, ECAP=ECAP, PW=PW)
    return meta, shared, cores


import concourse.bass as bass
import concourse.bacc as bacc
import concourse.mybir as mybir
from concourse import library_config
from concourse.tile import TileContext
from concourse.masks import make_identity
from concourse.bass_utils import run_bass_kernel_spmd

F16 = mybir.dt.float16
F32 = mybir.dt.float32
F8 = mybir.dt.float8e4
I16 = mybir.dt.int16
AX = mybir.AxisListType.X
ALU = mybir.AluOpType
AF = mybir.ActivationFunctionType


def build(meta):
    NBLK, NPC, ET, ECAP, PW = (meta[k] for k in ['NBLK', 'NPC', 'ET', 'ECAP', 'PW'])
    TPBb = meta['TPBb']
    toff = [0]
    for t in TPBb:
        toff.append(toff[-1] + t)
    nc = bacc.Bacc()

    dp = lambda n, s, d: nc.declare_dram_parameter(n, list(s), d, isOutput=False)
    x16 = dp('x16', [N, XROW], F16)
    xT_loc = dp('xT_loc', [128, NPC], F16)
    ix_x = dp('ix_x', [128, ECAP // 16], I16)
    ix_t2 = dp('ix_t2', [128, ECAP // 16], I16)
    ix_pool = dp('ix_pool', [128, GPC * PW // 16], I16)
    s8_d = dp('s8', [128, ET * 256], F8)
    enrm_d = dp('enrm', [128, ET], F16)
    mmean_d = dp('mmean', [NBLK, 128, GPC], F16)
    twin_d = dp('t_win', [32, GPC, 608], F16)
    bconv_rep = dp('bconv_rep', [GPC, 1], F32)
    wg1cat = dp('Wg1cat', [128, WAUG1], F16)
    w2ch = dp('W2chunks', [7, 128, W2AUG], F16)
    bg1ch = dp('bg1ch', [7, 128, 1], F16)
    as1f, ad1f = dp('as1f', [1, HF], F16), dp('ad1f', [1, HF], F16)
    as2f, ad2f = dp('as2f', [1, HF], F16), dp('ad2f', [1, HF], F16)
    wgcn = dp('wgcn_s', [128, F], F16)
    bgcn_col = dp('bgcn_col', [128, 1], F32)
    bgcn_row = dp('bgcn_row', [1, F], F32)
    bg2row = dp('bg2row', [1, HF], F16)
    wfg1p = dp('wfg1p', [14, 128, 128], F16)
    bfg1 = dp('bfg1', [1, 128], F32)
    wfg2p = dp('wfg2p', [2, 128, 128], F16)
    bfg2 = dp('bfg2', [1, 128], F32)
    wxtp = dp('wxtp', [5, 128, 256], F16)
    bxt = dp('bxt', [1, 256], F32)
    w1p = dp('w1p', [4, 128, 512], F16)
    b1 = dp('b1', [1, 512], F32)
    w2p = dp('w2p', [4, 128, 256], F16)
    b2 = dp('b2', [1, 256], F32)
    wop = dp('wop', [2, 128, 1], F16)
    bo_rep = dp('bo_rep', [GPC, 1], F32)
    wcol_d = dp('w_col', [32, 1], F16)
    wsel_d = dp('w_sel', [32, GPC, GPC], F16)

    out_d = nc.declare_dram_parameter('out', [GPC, 1], F32, isOutput=True)

    comb_shard = nc.dram_tensor('comb_shard', [NPC, CROW], F16)
    comb_full = nc.dram_tensor('comb_full', [8 * NPC, CROW], F16, addr_space="Shared")
    XFW = 1024
    xf_dram = nc.dram_tensor('xf_dram', [NPC, XFW], F16)

    RG = [list(range(8))]

    with TileContext(nc) as tc:
        nc.gpsimd.load_library(library_config.mlp)

        with tc.tile_pool(name="persist", bufs=1) as pp:
            w1aug_s = pp.tile([128, WAUG1], F16, tag="w1aug")
            nc.sync.dma_start(out=w1aug_s[:], in_=wg1cat[:])
            w2aug_s = pp.tile([128, 7, W2AUG], F16, tag="w2aug")
            for k in range(7):
                nc.sync.dma_start(out=w2aug_s[:, k, :], in_=w2ch[k])
            bg1_s = pp.tile([128, 7, 1], F16, tag="bg1")
            for k in range(7):
                nc.sync.dma_start(out=bg1_s[:, k, :], in_=bg1ch[k])
            a_s = pp.tile([128, 4, HF], F16, tag="aflat")
            for i, t in enumerate([as1f, ad1f, as2f, ad2f]):
                nc.sync.dma_start(out=a_s[:, i, :], in_=t[:].to_broadcast([128, HF]))
            xT_s = pp.tile([128, NPC], F16, tag="xT")
            nc.sync.dma_start(out=xT_s[:], in_=xT_loc[:])
            ixx_s = pp.tile([128, ECAP // 16], I16, tag="ixx")
            nc.sync.dma_start(out=ixx_s[:], in_=ix_x[:])
            ixt2_s = pp.tile([128, ECAP // 16], I16, tag="ixt2")
            nc.sync.dma_start(out=ixt2_s[:], in_=ix_t2[:])
            ixp_s = pp.tile([128, GPC * PW // 16], I16, tag="ixp")
            nc.sync.dma_start(out=ixp_s[:], in_=ix_pool[:])
            wgcn_s = pp.tile([128, F], F16, tag="wgcn")
            nc.sync.dma_start(out=wgcn_s[:], in_=wgcn[:])
            bgcnc_s = pp.tile([128, 1], F32, tag="bgcnc")
            nc.sync.dma_start(out=bgcnc_s[:], in_=bgcn_col[:])
            bgcnr_s = pp.tile([128, F], F32, tag="bgcnr")
            nc.sync.dma_start(out=bgcnr_s[:], in_=bgcn_row[:].to_broadcast([128, F]))
            bg2_s = pp.tile([128, HF], F16, tag="bg2")
            nc.sync.dma_start(out=bg2_s[:], in_=bg2row[:].to_broadcast([128, HF]))
            enrm_s = pp.tile([128, ET], F16, tag="enrm")
            nc.sync.dma_start(out=enrm_s[:], in_=enrm_d[:])
            mmean_s = pp.tile([128, NBLK, GPC], F16, tag="mmean")
            for b in range(NBLK):
                nc.sync.dma_start(out=mmean_s[:, b, :], in_=mmean_d[b])
            wcol_s = pp.tile([32, 1], F16, tag="wcol")
            nc.sync.dma_start(out=wcol_s[:], in_=wcol_d[:])
            wsel_s = pp.tile([32, GPC, GPC], F16, tag="wsel")
            nc.sync.dma_start(out=wsel_s[:], in_=wsel_d[:])
            bconv_s = pp.tile([GPC, 1], F32, tag="bconv")
            nc.sync.dma_start(out=bconv_s[:], in_=bconv_rep[:])
            wfg1_s = pp.tile([128, 14, 128], F16, tag="wfg1")
            for i in range(14):
                nc.sync.dma_start(out=wfg1_s[:, i, :], in_=wfg1p[i])
            wfg2_s = pp.tile([128, 2, 128], F16, tag="wfg2")
            for i in range(2):
                nc.sync.dma_start(out=wfg2_s[:, i, :], in_=wfg2p[i])
            wxt_s = pp.tile([128, 5, 256], F16, tag="wxt")
            for i in range(5):
                nc.sync.dma_start(out=wxt_s[:, i, :], in_=wxtp[i])
            w1_s = pp.tile([128, 4, 512], F16, tag="w1")
            for i in range(4):
                nc.sync.dma_start(out=w1_s[:, i, :], in_=w1p[i])
            w2_s = pp.tile([128, 4, 256], F16, tag="w2")
            for i in range(4):
                nc.sync.dma_start(out=w2_s[:, i, :], in_=w2p[i])
            wo_s = pp.tile([128, 2, 1], F16, tag="wo")
            for i in range(2):
                nc.sync.dma_start(out=wo_s[:, i, :], in_=wop[i])
            bias_s = {}
            for nm, t, w in [('bfg1', bfg1, 128), ('bfg2', bfg2, 128),
                             ('bxt', bxt, 256), ('b1', b1, 512), ('b2', b2, 256)]:
                bias_s[nm] = pp.tile([GPC, w], F32, tag="bias_" + nm, name="bias_" + nm)
                nc.sync.dma_start(out=bias_s[nm][:], in_=t[:].to_broadcast([GPC, w]))
            bo_s = pp.tile([GPC, 1], F32, tag="bo")
            nc.sync.dma_start(out=bo_s[:], in_=bo_rep[:])

            ident_s = pp.tile([128, 128], F16, tag="ident")
            make_identity(nc, ident_s[:])
            ones_s = pp.tile([1, 128], F16, tag="ones")
            nc.vector.memset(ones_s[:], 1.0)

            bd1_s = pp.tile([128, H], F16, tag="bd1")
            adst1_s = pp.tile([128, NBLK, H], F16, tag="adst1")
            adst2_s = pp.tile([128, NBLK, H], F16, tag="adst2")
            x1loc_s = pp.tile([128, NBLK, HF], F16, tag="x1loc")
            agg1_s = pp.tile([128, NBLK, F], F16, tag="agg1")
            c2_s = pp.tile([1, W2AUG], F16, tag="c2")
            # ex_ext per tile parity: [exp(10) | norm(1)]
            exA = pp.tile([128, H + 1], F16, tag="exA")
            exB = pp.tile([128, H + 1], F16, tag="exB")
            nc.vector.memset(exA[:], 1.0)
            nc.vector.memset(exB[:], 1.0)
            combst2 = pp.tile([128, 2, CROW], F16, tag="combst")
            nc.vector.memset(combst2[:], 0.0)
            xfst2 = pp.tile([128, 2, 1024], F16, tag="xfst")
            nc.vector.memset(xfst2[:], 0.0)

            # ---------------- B matrices
            with tc.tile_pool(name="bprep", bufs=2) as bp, \
                 tc.tile_pool(name="bprep_ps", bufs=2, space="PSUM") as bps:
                for i in range(2):   # B_s1 / B_d1 from Wg1
                    tmp = bp.tile([128, HF], F32, tag="btmp")
                    nc.vector.tensor_tensor(
                        out=tmp[:], in0=w1aug_s[:, 0:HF],
                        in1=a_s[:, i, :], op=ALU.mult)
                    red = bp.tile([128, H], F32, tag="bred")
                    nc.vector.tensor_reduce(
                        out=red[:], in_=tmp[:].rearrange("p (h f) -> p h f", h=H),
                        op=ALU.add, axis=AX)
                    if i == 0:
                        nc.vector.tensor_copy(out=w1aug_s[:, HF + F:WAUG1], in_=red[:])
                    else:
                        nc.vector.tensor_copy(out=bd1_s[:], in_=red[:])
                for k in range(7):
                    for i, col0 in [(2, HF), (3, HF + H)]:
                        tmp = bp.tile([128, HF], F32, tag="btmp")
                        nc.vector.tensor_tensor(
                            out=tmp[:], in0=w2aug_s[:, k, 0:HF],
                            in1=a_s[:, i, :], op=ALU.mult)
                        red = bp.tile([128, H], F32, tag="bred")
                        nc.vector.tensor_reduce(
                            out=red[:], in_=tmp[:].rearrange("p (h f) -> p h f", h=H),
                            op=ALU.add, axis=AX)
                        nc.vector.tensor_copy(out=w2aug_s[:, k, col0:col0 + H], in_=red[:])
                ps_c2 = bps.tile([1, W2AUG], F32, space="PSUM", tag="psc2")
                for k in range(7):
                    nc.tensor.matmul(out=ps_c2[:, 0:512], lhsT=bg1_s[:, k, :],
                                     rhs=w2aug_s[:, k, 0:512], start=(k == 0), stop=(k == 6))
                    nc.tensor.matmul(out=ps_c2[:, 512:W2AUG], lhsT=bg1_s[:, k, :],
                                     rhs=w2aug_s[:, k, 512:W2AUG], start=(k == 0), stop=(k == 6))
                nc.vector.tensor_copy(out=c2_s[:], in_=ps_c2[:])
                for b in range(NBLK):
                    ps_a = bps.tile([128, H], F32, space="PSUM", tag="psa")
                    nc.tensor.matmul(out=ps_a[:], lhsT=xT_s[:, 128 * b:128 * (b + 1)],
                                     rhs=bd1_s[:], start=True, stop=True)
                    nc.vector.tensor_copy(out=adst1_s[:, b, :], in_=ps_a[:])

            # ---------------- phase 1: GAT1 + GCN1
            with tc.tile_pool(name="p1", bufs=3) as p1, \
                 tc.tile_pool(name="p1g", bufs=2) as p1g, \
                 tc.tile_pool(name="p1s", bufs=2, space="PSUM") as p1s, \
                 tc.tile_pool(name="p1acc", bufs=1, space="PSUM") as p1acc:
                for b in range(NBLK):
                    TPB = TPBb[b]
                    EPB = TPB * 128
                    t0 = toff[b]
                    xgt = p1g.tile([128, 1, EPB], F16, tag="xgt")
                    nc.gpsimd.dma_gather(
                        out_ap=xgt[:], in_ap=x16[:],
                        idxs_ap=ixx_s[:, t0 * 8:(t0 + TPB) * 8],
                        num_idxs=EPB, num_idxs_reg=EPB, elem_size=XROW, transpose=True,
                        single_packet=False)
                    s8_b = p1g.tile([128, TPB, 256], F8, tag="s8b")
                    nc.sync.dma_start(out=s8_b[:], in_=s8_d[:, t0 * 256:(t0 + TPB) * 256])
                    ps_out = p1acc.tile([128, EXW], F32, space="PSUM", tag="psout", name="psout")[:]
                    for k in range(TPB):
                        s01_t = s8_b[:, k, 0:128]
                        s01t_t = s8_b[:, k, 128:256]
                        lhs = xgt[:, 0, 128 * k:128 * (k + 1)]
                        ps1 = p1s.tile([128, WAUG1], F32, space="PSUM", tag="ps1")
                        nc.tensor.matmul(out=ps1[:, 0:512], lhsT=lhs,
                                         rhs=w1aug_s[:, 0:512], start=True, stop=True)
                        nc.tensor.matmul(out=ps1[:, 512:WAUG1], lhsT=lhs,
                                         rhs=w1aug_s[:, 512:WAUG1], start=True, stop=False)
                        nc.tensor.matmul(out=ps1[:, HF + F:WAUG1],
                                         lhsT=s01t_t, rhs=adst1_s[:, b, :],
                                         start=False, stop=True)
                        ex = exA if k % 2 == 0 else exB
                        lr02 = p1.tile([128, H], F32, tag="lr02")
                        nc.scalar.activation(out=lr02[:], in_=ps1[:, HF + F:WAUG1],
                                             func=AF.Copy, scale=0.2)
                        lr = p1.tile([128, H], F32, tag="lr")
                        nc.vector.tensor_tensor(out=lr[:], in0=ps1[:, HF + F:WAUG1],
                                                in1=lr02[:], op=ALU.max)
                        nc.scalar.activation(out=ex[:, 0:H], in_=lr[:], func=AF.Exp)
                        nc.vector.tensor_copy(out=ex[:, H:H + 1],
                                              in_=enrm_s[:, t0 + k:t0 + k + 1])
                        exv = p1.tile([128, EXW], F16, tag="exv")
                        if k % 4 != 3:
                            h1sb = p1.tile([128, HF + F], F16, tag="h1sb")
                            nc.scalar.activation(out=h1sb[:], in_=ps1[:, 0:HF + F],
                                                 func=AF.Copy)
                            nc.vector.tensor_tensor(
                                out=exv[:, 0:HF + F].rearrange("p (h f) -> p h f", h=H + 1),
                                in0=h1sb[:].rearrange("p (h f) -> p h f", h=H + 1),
                                in1=ex[:, :, None].to_broadcast([128, H + 1, F]),
                                op=ALU.mult)
                        else:
                            nc.vector.tensor_tensor(
                                out=exv[:, 0:HF + F].rearrange("p (h f) -> p h f", h=H + 1),
                                in0=ps1[:, 0:HF + F].rearrange("p (h f) -> p h f", h=H + 1),
                                in1=ex[:, :, None].to_broadcast([128, H + 1, F]),
                                op=ALU.mult)
                        nc.vector.tensor_copy(out=exv[:, HF + F:EXW], in_=ex[:, 0:H])
                        nc.tensor.matmul(out=ps_out[:, 0:512], lhsT=s01_t,
                                         rhs=exv[:, 0:512], start=(k == 0), stop=(k == TPB - 1))
                        nc.tensor.matmul(out=ps_out[:, 512:EXW], lhsT=s01_t,
                                         rhs=exv[:, 512:EXW], start=(k == 0), stop=(k == TPB - 1))
                    rec = p1.tile([128, H], F32, tag="rec")
                    nc.vector.reciprocal(out=rec[:], in_=ps_out[:, HF + F:EXW])
                    nc.vector.tensor_tensor(
                        out=x1loc_s[:, b, :].rearrange("p (h f) -> p h f", h=H),
                        in0=ps_out[:, 0:HF].rearrange("p (h f) -> p h f", h=H),
                        in1=rec[:, :, None].to_broadcast([128, H, F]),
                        op=ALU.mult)
                    nc.vector.tensor_copy(out=agg1_s[:, b, :], in_=ps_out[:, HF:HF + F])

            # ---------------- phase 2: comb table build + collective + conv
            with tc.tile_pool(name="p2", bufs=2) as p2:
              with tc.tile_pool(name="p2sa", bufs=2, space="PSUM") as p2s, \
                   tc.tile_pool(name="p2ta", bufs=2, space="PSUM") as p2t:
                x1t_s = p2.tile([128, 7, NPC], F16, tag="x1t", bufs=1)
                nc.vector.memset(x1t_s[:], 0.0)
                for b in range(NBLK):
                    for fb in range(7):
                        c0, c1 = 128 * fb, min(128 * (fb + 1), HF)
                        psT = p2t.tile([128, 128], F16, space="PSUM", tag="psT")
                        nc.tensor.transpose(out=psT[:c1 - c0, :],
                                            in_=x1loc_s[:, b, c0:c1],
                                            identity=ident_s[:])
                        nc.vector.tensor_copy(
                            out=x1t_s[0:c1 - c0, fb, 128 * b:128 * (b + 1)],
                            in_=psT[:c1 - c0, :])
                for b in range(NBLK):
                    combst = combst2[:, b % 2, :]
                    # x2 for block b
                    psT = p2t.tile([128, 128], F16, space="PSUM", tag="psT")
                    nc.tensor.transpose(out=psT[:F, :], in_=agg1_s[:, b, :],
                                        identity=ident_s[:])
                    x2lt = p2.tile([128, 128], F16, tag="x2lt")
                    nc.vector.tensor_scalar(out=x2lt[:F, :], in0=psT[:F, :],
                                            scalar1=bgcnc_s[:F, :], scalar2=None,
                                            op0=ALU.add)
                    ps_x2 = p2s.tile([128, F], F32, space="PSUM", tag="psx2")
                    nc.tensor.matmul(out=ps_x2[:], lhsT=x2lt[:F, :], rhs=wgcn_s[:F, :],
                                     start=True, stop=True)
                    nc.vector.tensor_copy(out=combst[:, HF:HF + F], in_=ps_x2[:])
                    # T2 for block b
                    ps_t2 = p2s.tile([128, W2AUG], F32, space="PSUM", tag="pst2")
                    for k in range(7):
                        nc.tensor.matmul(out=ps_t2[:, 0:512],
                                         lhsT=x1t_s[:, k, 128 * b:128 * (b + 1)],
                                         rhs=w2aug_s[:, k, 0:512], start=(k == 0), stop=False)
                        nc.tensor.matmul(out=ps_t2[:, 512:W2AUG],
                                         lhsT=x1t_s[:, k, 128 * b:128 * (b + 1)],
                                         rhs=w2aug_s[:, k, 512:W2AUG], start=(k == 0), stop=False)
                    nc.tensor.matmul(out=ps_t2[:, 0:512], lhsT=ones_s[:],
                                     rhs=c2_s[:, 0:512], start=False, stop=True)
                    nc.tensor.matmul(out=ps_t2[:, 512:W2AUG], lhsT=ones_s[:],
                                     rhs=c2_s[:, 512:W2AUG], start=False, stop=True)
                    nc.vector.tensor_copy(out=combst[:, 0:HF], in_=ps_t2[:, 0:HF])
                    nc.vector.tensor_copy(out=combst[:, HF + F:WAUG1],
                                          in_=ps_t2[:, HF:HF + H])
                    nc.vector.tensor_copy(out=adst2_s[:, b, :], in_=ps_t2[:, HF + H:W2AUG])
                    nc.sync.dma_start(out=comb_shard[128 * b:128 * (b + 1), :],
                                      in_=combst)
                nc.gpsimd.collective_compute(
                    "AllGather", ALU.bypass, replica_groups=RG,
                    ins=[comb_shard[:]], outs=[comb_full[:]])

              with tc.tile_pool(name="p2sc", bufs=1, space="PSUM") as p2s, \
                   tc.tile_pool(name="p2tc", bufs=2, space="PSUM") as p2t:
                # conv branch overlaps the collective
                twin_s = p2.tile([32, GPC, 608], F16, tag="twin", bufs=1)
                nc.sync.dma_start(out=twin_s[:], in_=twin_d[:])
                ps_ya = p2s.tile([GPC, 512], F32, space="PSUM", tag="psya")
                ps_yb = p2s.tile([GPC, 96], F32, space="PSUM", tag="psyb")
                for g in range(GPC):
                    nc.tensor.matmul(out=ps_ya[:], lhsT=wsel_s[:, g, :],
                                     rhs=twin_s[:, g, 0:512], start=(g == 0), stop=(g == GPC - 1))
                    nc.tensor.matmul(out=ps_yb[:], lhsT=wsel_s[:, g, :],
                                     rhs=twin_s[:, g, 512:608], start=(g == 0), stop=(g == GPC - 1))
                y_s = p2.tile([GPC, 608], F16, tag="ys")
                nc.vector.tensor_scalar(out=y_s[:, 0:512], in0=ps_ya[:],
                                        scalar1=bconv_s[:], scalar2=0.0,
                                        op0=ALU.add, op1=ALU.max)
                nc.vector.tensor_scalar(out=y_s[:, 512:608], in0=ps_yb[:],
                                        scalar1=bconv_s[:], scalar2=0.0,
                                        op0=ALU.add, op1=ALU.max)
                yt_s = pp.tile([128, 5, GPC], F16, tag="yt")
                nc.vector.memset(yt_s[:], 0.0)
                for i in range(5):
                    c0, c1 = 128 * i, min(128 * (i + 1), 608)
                    psT = p2t.tile([128, 128], F16, space="PSUM", tag="psT")
                    nc.tensor.transpose(out=psT[:c1 - c0, :GPC], in_=y_s[:, c0:c1],
                                        identity=ident_s[:GPC, :GPC])
                    nc.vector.tensor_copy(out=yt_s[0:c1 - c0, i, :], in_=psT[:c1 - c0, :GPC])
                ps_xt = p2s.tile([GPC, 256], F32, space="PSUM", tag="psxt")
                for i in range(5):
                    nc.tensor.matmul(out=ps_xt[:], lhsT=yt_s[:, i, :], rhs=wxt_s[:, i, :],
                                     start=(i == 0), stop=(i == 4))
                xt_s = p2.tile([GPC, 256], F16, tag="xts")
                nc.vector.tensor_tensor(out=xt_s[:], in0=ps_xt[:],
                                        in1=bias_s['bxt'][:], op=ALU.add)
                xtT_s = pp.tile([128, 2, GPC], F16, tag="xtT")
                for i in range(2):
                    psT = p2t.tile([128, 128], F16, space="PSUM", tag="psT")
                    nc.tensor.transpose(out=psT[:, :GPC], in_=xt_s[:, 128 * i:128 * (i + 1)],
                                        identity=ident_s[:GPC, :GPC])
                    nc.vector.tensor_copy(out=xtT_s[:, i, :], in_=psT[:, :GPC])

            # ---------------- phase 3: GAT2 + GCN2 (+ inline mean pool)
            with tc.tile_pool(name="p3", bufs=3) as p3, \
                 tc.tile_pool(name="p3g", bufs=2) as p3g, \
                 tc.tile_pool(name="p3s", bufs=2, space="PSUM") as p3s, \
                 tc.tile_pool(name="p3acc", bufs=1, space="PSUM") as p3acc, \
                 tc.tile_pool(name="p3m", bufs=1, space="PSUM") as p3m:
                ps_m = p3m.tile([GPC, HF + F], F32, space="PSUM", tag="psm", name="psm")[:]
                for b in range(NBLK):
                    TPB = TPBb[b]
                    EPB = TPB * 128
                    t0 = toff[b]
                    xfst = xfst2[:, b % 2, :]
                    v2g = p3g.tile([128, TPB, CROW], F16, tag="v2g")
                    nc.gpsimd.dma_gather(
                        out_ap=v2g[:], in_ap=comb_full[:],
                        idxs_ap=ixt2_s[:, t0 * 8:(t0 + TPB) * 8],
                        num_idxs=EPB, num_idxs_reg=EPB, elem_size=CROW,
                        single_packet=False)
                    s8_b = p3g.tile([128, TPB, 256], F8, tag="s8b3")
                    nc.sync.dma_start(out=s8_b[:], in_=s8_d[:, t0 * 256:(t0 + TPB) * 256])
                    ps_out = p3acc.tile([128, EXW], F32, space="PSUM", tag="psout3", name="psout3")[:]
                    for k in range(TPB):
                        s01_t = s8_b[:, k, 0:128]
                        s01t_t = s8_b[:, k, 128:256]
                        ps_l = p3s.tile([128, H], F32, space="PSUM", tag="psl")
                        nc.tensor.matmul(out=ps_l[:], lhsT=s01t_t,
                                         rhs=adst2_s[:, b, :], start=True, stop=False)
                        nc.tensor.matmul(out=ps_l[:], lhsT=ident_s[:],
                                         rhs=v2g[:, k, HF + F:WAUG1], start=False, stop=True)
                        ex = exA if k % 2 == 0 else exB
                        lr02 = p3.tile([128, H], F32, tag="lr023")
                        nc.scalar.activation(out=lr02[:], in_=ps_l[:], func=AF.Copy, scale=0.2)
                        lr = p3.tile([128, H], F32, tag="lr3")
                        nc.vector.tensor_tensor(out=lr[:], in0=ps_l[:], in1=lr02[:], op=ALU.max)
                        nc.scalar.activation(out=ex[:, 0:H], in_=lr[:], func=AF.Exp)
                        nc.vector.tensor_copy(out=ex[:, H:H + 1],
                                              in_=enrm_s[:, t0 + k:t0 + k + 1])
                        exv = p3.tile([128, EXW], F16, tag="exv3")
                        nc.vector.tensor_tensor(
                            out=exv[:, 0:HF + F].rearrange("p (h f) -> p h f", h=H + 1),
                            in0=v2g[:, k, 0:HF + F].rearrange("p (h f) -> p h f", h=H + 1),
                            in1=ex[:, :, None].to_broadcast([128, H + 1, F]),
                            op=ALU.mult)
                        nc.vector.tensor_copy(out=exv[:, HF + F:EXW], in_=ex[:, 0:H])
                        nc.tensor.matmul(out=ps_out[:, 0:512], lhsT=s01_t,
                                         rhs=exv[:, 0:512], start=(k == 0), stop=(k == TPB - 1))
                        nc.tensor.matmul(out=ps_out[:, 512:EXW], lhsT=s01_t,
                                         rhs=exv[:, 512:EXW], start=(k == 0), stop=(k == TPB - 1))
                    rec = p3.tile([128, H], F32, tag="rec3")
                    nc.vector.reciprocal(out=rec[:], in_=ps_out[:, HF + F:EXW])
                    u_s = p3.tile([128, HF], F16, tag="us")
                    nc.vector.tensor_tensor(
                        out=u_s[:].rearrange("p (h f) -> p h f", h=H),
                        in0=ps_out[:, 0:HF].rearrange("p (h f) -> p h f", h=H),
                        in1=rec[:, :, None].to_broadcast([128, H, F]),
                        op=ALU.mult)
                    v_s = p3.tile([128, HF], F16, tag="vs")
                    nc.vector.tensor_tensor(out=v_s[:], in0=u_s[:], in1=bg2_s[:],
                                            op=ALU.add)
                    nc.scalar.activation(out=xfst[:, 0:HF], in_=v_s[:], func=AF.Relu)
                    g2f = p3.tile([128, F], F32, tag="g2f")
                    nc.vector.tensor_tensor(out=g2f[:], in0=ps_out[:, HF:HF + F],
                                            in1=bgcnr_s[:], op=ALU.add)
                    nc.scalar.activation(out=xfst[:, HF:HF + F], in_=g2f[:], func=AF.Relu)
                    nc.scalar.activation(out=xfst[:, 896:896 + F], in_=g2f[:], func=AF.Relu)
                    nc.sync.dma_start(out=xf_dram[128 * b:128 * (b + 1), :], in_=xfst[:])
                    nc.tensor.matmul(out=ps_m[:, 0:512], lhsT=mmean_s[:, b, :],
                                     rhs=xfst[:, 0:512], start=(b == 0),
                                     stop=(b == NBLK - 1))
                    nc.tensor.matmul(out=ps_m[:, 512:HF + F], lhsT=mmean_s[:, b, :],
                                     rhs=xfst[:, 512:HF + F], start=(b == 0),
                                     stop=(b == NBLK - 1))
                mean_s = pp.tile([GPC, HF + F], F16, tag="means")
                nc.vector.tensor_copy(out=mean_s[:], in_=ps_m[:])

            # ---------------- phase 4: max pool + head
            with tc.tile_pool(name="p4", bufs=2) as p4:
              with tc.tile_pool(name="p4s", bufs=1, space="PSUM") as p4s:
                gmax1T = pp.tile([128, 7, GPC], F16, tag="gmax1T")
                nc.vector.memset(gmax1T[:], 0.0)
                gmax2T = pp.tile([128, 1, GPC], F16, tag="gmax2T")
                nc.vector.memset(gmax2T[:], 0.0)
                CH = GPC // 2
                for h in range(2):
                    slab = p4.tile([128, 7, CH * PW], F16, tag="slab")
                    nc.gpsimd.dma_gather(
                        out_ap=slab[:], in_ap=xf_dram[:, 0:CROW],
                        idxs_ap=ixp_s[:, h * (CH * PW // 16):(h + 1) * (CH * PW // 16)],
                        num_idxs=CH * PW, num_idxs_reg=CH * PW, elem_size=CROW,
                        elem_step=1024, transpose=True, single_packet=False)
                    slab2 = p4.tile([128, 1, CH * PW], F16, tag="slab2")
                    nc.gpsimd.dma_gather(
                        out_ap=slab2[:], in_ap=xf_dram[:, 896:1024],
                        idxs_ap=ixp_s[:, h * (CH * PW // 16):(h + 1) * (CH * PW // 16)],
                        num_idxs=CH * PW, num_idxs_reg=CH * PW, elem_size=XROW,
                        elem_step=1024, transpose=True, single_packet=False)
                    for g in range(CH):
                        for j in range(7):
                            nc.vector.tensor_reduce(
                                out=gmax1T[:, j, h * CH + g:h * CH + g + 1],
                                in_=slab[:, j, g * PW:(g + 1) * PW],
                                op=ALU.max, axis=AX)
                        nc.vector.tensor_reduce(
                            out=gmax2T[:, 0, h * CH + g:h * CH + g + 1],
                            in_=slab2[:, 0, g * PW:(g + 1) * PW],
                            op=ALU.max, axis=AX)
              with tc.tile_pool(name="p4sh", bufs=1, space="PSUM") as p4s:
                gmean1T = pp.tile([128, 7, GPC], F16, tag="gmean1T")
                nc.vector.memset(gmean1T[:], 0.0)
                gmean2T = pp.tile([128, 1, GPC], F16, tag="gmean2T")
                nc.vector.memset(gmean2T[:], 0.0)
                for i in range(7):
                    c0, c1 = 128 * i, min(128 * (i + 1), HF)
                    psT = p4s.tile([128, 128], F16, space="PSUM", tag="psT4", bufs=2)
                    nc.tensor.transpose(out=psT[:c1 - c0, :GPC], in_=mean_s[:, c0:c1],
                                        identity=ident_s[:GPC, :GPC])
                    nc.vector.tensor_copy(out=gmean1T[0:c1 - c0, i, :], in_=psT[:c1 - c0, :GPC])
                psT = p4s.tile([128, 128], F16, space="PSUM", tag="psT4", bufs=2)
                nc.tensor.transpose(out=psT[:F, :GPC], in_=mean_s[:, HF:HF + F],
                                    identity=ident_s[:GPC, :GPC])
                nc.vector.tensor_copy(out=gmean2T[0:F, 0, :], in_=psT[:F, :GPC])

                def head_mm(ps, chunks, rhs_tile, nw):
                    n = len(chunks)
                    for i, ch in enumerate(chunks):
                        nc.tensor.matmul(out=ps[:], lhsT=ch, rhs=rhs_tile[:, i, :nw],
                                         start=(i == 0), stop=(i == n - 1))

                def bias_relu_T(ps, bias_ap, w, relu, nT, tagb):
                    zs = p4.tile([GPC, w], F16, tag="z" + tagb)
                    nc.vector.tensor_tensor(out=zs[:], in0=ps[:], in1=bias_ap, op=ALU.add)
                    if relu:
                        nc.vector.tensor_scalar(out=zs[:], in0=zs[:], scalar1=0.0,
                                                scalar2=None, op0=ALU.max)
                    zT = pp.tile([128, nT, GPC], F16, tag="zT" + tagb)
                    for i in range(nT):
                        psT2 = p4s.tile([128, 128], F16, space="PSUM", tag="psT4", bufs=2)
                        nc.tensor.transpose(out=psT2[:, :GPC], in_=zs[:, 128 * i:128 * (i + 1)],
                                            identity=ident_s[:GPC, :GPC])
                        nc.vector.tensor_copy(out=zT[:, i, :], in_=psT2[:, :GPC])
                    return zT

                ps_z1 = p4s.tile([GPC, 128], F32, space="PSUM", tag="psz1")
                head_mm(ps_z1, [gmax1T[:, j, :] for j in range(7)]
                        + [gmean1T[:, j, :] for j in range(7)], wfg1_s, 128)
                z1T = bias_relu_T(ps_z1, bias_s['bfg1'][:], 128, True, 1, "1")
                ps_z2 = p4s.tile([GPC, 128], F32, space="PSUM", tag="psz2")
                head_mm(ps_z2, [gmax2T[:, 0, :], gmean2T[:, 0, :]], wfg2_s, 128)
                z2T = bias_relu_T(ps_z2, bias_s['bfg2'][:], 128, True, 1, "2")
                ps_h1 = p4s.tile([GPC, 512], F32, space="PSUM", tag="psh1")
                head_mm(ps_h1, [z1T[:, 0, :], z2T[:, 0, :], xtT_s[:, 0, :], xtT_s[:, 1, :]],
                        w1_s, 512)
                h1T = bias_relu_T(ps_h1, bias_s['b1'][:], 512, True, 4, "h1")
                ps_h2 = p4s.tile([GPC, 256], F32, space="PSUM", tag="psh2")
                head_mm(ps_h2, [h1T[:, i, :] for i in range(4)], w2_s, 256)
                h2T = bias_relu_T(ps_h2, bias_s['b2'][:], 256, True, 2, "h2")
                ps_o = p4s.tile([GPC, 1], F32, space="PSUM", tag="pso")
                head_mm(ps_o, [h2T[:, i, :] for i in range(2)], wo_s, 1)
                o_s = p4.tile([GPC, 1], F32, tag="os")
                nc.vector.tensor_scalar(out=o_s[:], in0=ps_o[:], scalar1=bo_s[:],
                                        scalar2=None, op0=ALU.add)
                nc.sync.dma_start(out=out_d[:], in_=o_s[:])

    nc.compile()
    return nc


def build_in_maps(nc, shared, cores):
    declared = set()
    import concourse.mybir as _mb
    for alloc in nc.m.functions[0].allocations:
        if isinstance(alloc, _mb.MemoryLocationSet) and alloc.kind == "ExternalInput":
            declared.add(alloc.memorylocations[0].name)
    in_maps = []
    for c in range(8):
        m = dict(shared)
        m.update(cores[c])
        in_maps.append({k: np.ascontiguousarray(v) for k, v in m.items()
                        if k in declared})
    return in_maps


_CACHE = {}


def run_device(inputs):
    meta, shared, cores = prep(**inputs)
    key = (meta['NBLK'], meta['TPBb'], meta['PW'])
    if key not in _CACHE:
        _CACHE[key] = build(meta)
    nc = _CACHE[key]
    in_maps = build_in_maps(nc, shared, cores)
    res = run_bass_kernel_spmd(nc, in_maps, core_ids=list(range(8)))
    out = np.concatenate([res.results[c]['out'] for c in range(8)], axis=0)
    return out.astype(np.float32)


def kernel(**inputs):
    return run_device(inputs)


# revision 10
# speedup vs baseline: 1.1112x; 1.0357x over previous
"""Trainium2 Bass kernel for nn_GAT_GCN (gnn_message_passing), 8 NeuronCores.

v2 strategy (from v1 baseline at 1075us):
 - Dst-node sharding, graph-aligned (16 graphs/core). Within a core, nodes
   are BIN-PACKED into 128-node blocks balancing per-block edge counts, so
   every block needs the same tile count (TPB~9 vs v1's 10) -> 153 tiles
   instead of 170.
 - One merged exchange row per node: [h2(780) | x2(78) | asrc2(10) | pad]
   = 896 f16 = 1792B. One AllGather (31.2MB) and ONE phase-3 gather per
   edge (v1 had two gathers + 1024-col rows).
 - Scatter matrices s01/s01t in fp8 (exact 0/1); GCN's norm is applied via
   an extra broadcast column in the exp multiply (ex_ext[:,10]=norm_e), so
   the separate snrm matrix is gone.
 - Per-edge scale multiply (exv) runs in DVE 2x mode: PSUM->SBUF fp16 copy
   on ACT for most tiles, direct-PSUM for the rest (engine balance).
 - Leakyrelu max on gpsimd; denominators folded into the single scatter
   matmul pair (rhs = [exv(858) | ex(10)]).
 - Mean-pool accumulated inline in phase 3 (PE matmul); max-pool gathers
   read one merged x1f|x2f table; reduces split DVE/gpsimd.
"""
import sys
sys.path.insert(0, '/opt/trn_rl_repo')
import numpy as np
import ml_dtypes

N, E, G, F, H = 16384, 131072, 128, 78, 10
NCORE, GPC = 8, 16
HF = H * F                  # 780
WAUG1 = HF + F + H          # 868 = h1(780) | hgcn(78) | asrc(10)
W2AUG = HF + 2 * H          # 800 = h2 | asrc2 | adst2
CROW = 896                  # comb row: h2(780) | x2(78) | asrc2(10) | pad
XROW = 128                  # x table row, fp16 (256B)
EXW = WAUG1                 # 868 = exv(858) | ex(10) scatter rhs width


def _wrap16(v):
    v = np.asarray(v, np.int16)
    assert len(v) % 16 == 0
    m = v.reshape(-1, 16).T
    return np.tile(m, (8, 1)).copy()


def _f16(a):
    return np.ascontiguousarray(np.asarray(a, np.float32)).astype(np.float16)


def _f8(a):
    return np.ascontiguousarray(np.asarray(a, np.float32)).astype(ml_dtypes.float8_e4m3)


def prep(x, edge_index, batch, target, Wg1, as1, ad1, bg1, Wg2, as2, ad2, bg2,
         Wgcn, bgcn, Wfg1, bfg1, Wfg2, bfg2, wconv, bconv, Wxt, bxt,
         W1, b1, W2, b2, Wo, bo):
    x = np.asarray(x, np.float32)
    ei = np.asarray(edge_index, np.int64)
    batch = np.asarray(batch, np.int64)
    target = np.asarray(target, np.float32)

    loops = np.arange(N, dtype=np.int64)
    src = np.concatenate([ei[0], loops])
    dst = np.concatenate([ei[1], loops])

    counts = np.bincount(batch, minlength=G)
    node_off = np.concatenate([[0], np.cumsum(counts)])
    n_lo = node_off[np.arange(NCORE) * GPC]
    n_hi = node_off[(np.arange(NCORE) + 1) * GPC]

    deg = np.bincount(dst, minlength=N).astype(np.float64)
    dinv = 1.0 / np.sqrt(deg)
    norm = (dinv[src] * dinv[dst]).astype(np.float32)

    Lmax = int((n_hi - n_lo).max())
    NBLK = (Lmax + 127) // 128
    NPC = NBLK * 128
    assert NCORE * NPC < 32768

    # ---- per-core bin packing of nodes into NBLK blocks of <=128 nodes,
    # balancing per-block edge (degree) sums.
    blk_of = np.zeros(N, np.int64)     # block index of node (within its core)
    slot_of = np.zeros(N, np.int64)    # slot within block
    tiles_cb = np.zeros((NCORE, NBLK), np.int64)
    for c in range(NCORE):
        ids = np.arange(n_lo[c], n_hi[c])
        degs = deg[ids]
        order = np.argsort(-degs, kind='stable')
        bins_e = np.zeros(NBLK)
        bins_n = np.zeros(NBLK, np.int64)
        for i in order:
            cand = np.where(bins_n < 128, bins_e, np.inf)
            bsel = int(np.argmin(cand))
            nid = ids[i]
            blk_of[nid] = bsel
            slot_of[nid] = bins_n[bsel]
            bins_e[bsel] += degs[i]
            bins_n[bsel] += 1
        tiles_cb[c] = (bins_e.astype(np.int64) + 127) // 128
    TPBb = tiles_cb.max(axis=0)        # per-block tile count (same all cores)
    toff = np.concatenate([[0], np.cumsum(TPBb)])
    ET = int(toff[-1])
    ECAP = ET * 128
    node_owner = np.searchsorted(n_hi - 1, np.arange(N), side='left')
    node_owner = np.minimum(node_owner, NCORE - 1)
    local_id = blk_of * 128 + slot_of                    # 0..NPC-1
    pad_gid = node_owner * NPC + local_id

    # edges sorted by (core, block)
    ecore = node_owner[dst]
    eblk = blk_of[dst]
    ekey = ecore * NBLK + eblk
    order = np.argsort(ekey, kind='stable')
    srcs, dsts = src[order], dst[order]
    norms = norm[order]
    ekey_s = ekey[order]

    PW = int(np.ceil(counts.max() / 16) * 16)

    cores = []
    for c in range(NCORE):
        esrc = np.zeros(ECAP, np.int64)
        s01 = np.zeros((ET, 128, 128), ml_dtypes.float8_e4m3)
        s01t = np.zeros((ET, 128, 128), ml_dtypes.float8_e4m3)
        enrm = np.zeros((ET, 128), np.float16)    # [tile, edge-slot] norm
        for b in range(NBLK):
            lo = np.searchsorted(ekey_s, c * NBLK + b)
            hi = np.searchsorted(ekey_s, c * NBLK + b, side='right')
            ne = hi - lo
            t0 = toff[b]
            if ne > 0:
                j = np.arange(ne)
                t_loc = j // 128
                e_loc = j % 128
                ld = slot_of[dsts[lo:hi]]
                gslot = (t0 + t_loc) * 128 + e_loc
                esrc[gslot] = srcs[lo:hi]
                s01[t0 + t_loc, e_loc, ld] = 1.0
                s01t[t0 + t_loc, ld, e_loc] = 1.0
                enrm[t0 + t_loc, e_loc] = norms[lo:hi].astype(np.float16)
            # pad dst slots (no nodes) get a fake denominator entry
            nnode = int(((node_owner == c) & (blk_of == b)).sum()) if False else None
        # count nodes per (c, b) to set fake denominators on empty slots
        nb = np.zeros(NBLK, np.int64)
        sel = np.arange(n_lo[c], n_hi[c])
        for b in range(NBLK):
            nb[b] = int((blk_of[sel] == b).sum())
        for b in range(NBLK):
            if nb[b] < 128:
                s01[toff[b], 0, nb[b]:] = 1.0

        # pooling indices (local node ids into xf table)
        pool_idx = np.zeros(GPC * PW, np.int64)
        for g in range(GPC):
            gg = c * GPC + g
            ids = np.arange(node_off[gg], node_off[gg + 1])
            lid = local_id[ids]
            cnt = len(ids)
            pool_idx[g * PW:g * PW + cnt] = lid
            pool_idx[g * PW + cnt:(g + 1) * PW] = lid[0]
        mmean = np.zeros((NBLK, 128, GPC), np.float16)
        for g in range(GPC):
            gg = c * GPC + g
            ids = np.arange(node_off[gg], node_off[gg + 1])
            mmean[blk_of[ids], slot_of[ids], g] = np.float16(1.0 / len(ids))

        t_win = np.zeros((32, GPC, 608), np.float16)
        tg = target[c * GPC:(c + 1) * GPC, 0, :]
        for k in range(32):
            t_win[k, :, :594] = tg[:, k:k + 594].astype(np.float16)

        xT = np.zeros((128, NPC), np.float16)
        ids = np.arange(n_lo[c], n_hi[c])
        xT[:F, local_id[ids]] = x[ids, :].T.astype(np.float16)

        # combined fp8 s-matrix stream: [partition, tile*(s01|s01t)]
        s8 = np.zeros((128, ET, 256), ml_dtypes.float8_e4m3)
        s8[:, :, 0:128] = s01.transpose(1, 0, 2)
        s8[:, :, 128:256] = s01t.transpose(1, 0, 2)
        s8 = s8.reshape(128, ET * 256)
        cores.append(dict(
            ix_x=_wrap16(esrc),
            ix_t2=_wrap16(pad_gid[esrc]),
            ix_pool=_wrap16(pool_idx),
            s8=s8, enrm=np.ascontiguousarray(enrm.T.astype(np.float32)),
            mmean=mmean, t_win=t_win, xT_loc=xT,
            bconv_rep=np.full((GPC, 1), float(bconv[0]), np.float32),
        ))

    x16 = np.zeros((N, XROW), np.float16)
    x16[:, :F] = x.astype(np.float16)

    Wg1cat = np.zeros((128, WAUG1), np.float16)
    Wg1cat[:F, :HF] = _f16(Wg1)
    Wg1cat[:F, HF:HF + F] = _f16(Wgcn)
    W2chunks = np.zeros((7, 128, W2AUG), np.float16)
    for k in range(7):
        r0, r1 = 128 * k, min(128 * (k + 1), HF)
        W2chunks[k, :r1 - r0, :HF] = _f16(Wg2[r0:r1, :])
    bg1ch = np.zeros((7, 128, 1), np.float16)
    bg1f = _f16(bg1).reshape(-1)
    for k in range(7):
        r0, r1 = 128 * k, min(128 * (k + 1), HF)
        bg1ch[k, :r1 - r0, 0] = bg1f[r0:r1]

    def pack_rows(Wm, splits, ncol):
        out = np.zeros((len(splits), 128, ncol), np.float16)
        for i, (r0, r1) in enumerate(splits):
            out[i, :r1 - r0, :] = _f16(Wm[r0:r1, :])
        return out

    sp7 = [(128 * i, min(128 * (i + 1), HF)) for i in range(7)]
    wfg1p = np.concatenate([pack_rows(Wfg1[:HF], sp7, 128),
                            pack_rows(Wfg1[HF:], sp7, 128)], axis=0)
    wfg2p = pack_rows(Wfg2, [(0, F), (F, 2 * F)], 128)
    wxtp = pack_rows(Wxt, [(128 * i, min(128 * (i + 1), 594)) for i in range(5)], 256)
    w1p = pack_rows(W1, [(128 * i, 128 * (i + 1)) for i in range(4)], 512)
    w2p = pack_rows(W2, [(128 * i, 128 * (i + 1)) for i in range(4)], 256)
    wop = pack_rows(Wo, [(0, 128), (128, 256)], 1)

    wgcn_s = np.zeros((128, F), np.float16)
    wgcn_s[:F] = _f16(Wgcn)
    bgcn_col = np.zeros((128, 1), np.float32)
    bgcn_col[:F, 0] = np.asarray(bgcn, np.float32)

    shared = dict(
        x16=x16, Wg1cat=Wg1cat, W2chunks=W2chunks, bg1ch=bg1ch,
        as1f=_f16(as1).reshape(1, HF), ad1f=_f16(ad1).reshape(1, HF),
        as2f=_f16(as2).reshape(1, HF), ad2f=_f16(ad2).reshape(1, HF),
        wgcn_s=wgcn_s, bgcn_col=bgcn_col,
        bgcn_row=np.asarray(bgcn, np.float32).reshape(1, F),
        bg2row=_f16(bg2).reshape(1, HF),
        wfg1p=wfg1p, bfg1=np.asarray(bfg1, np.float32).reshape(1, 128),
        wfg2p=wfg2p, bfg2=np.asarray(bfg2, np.float32).reshape(1, 128),
        wxtp=wxtp, bxt=np.asarray(bxt, np.float32).reshape(1, 256),
        w1p=w1p, b1=np.asarray(b1, np.float32).reshape(1, 512),
        w2p=w2p, b2=np.asarray(b2, np.float32).reshape(1, 256),
        wop=wop, bo_rep=np.full((GPC, 1), float(np.asarray(bo).reshape(-1)[0]), np.float32),
        w_col=np.zeros((32, 1), np.float16),
        w_sel=np.zeros((32, GPC, GPC), np.float16),
    )
    shared['w_col'][:, 0] = _f16(np.asarray(wconv).reshape(-1))
    for g in range(GPC):
        shared['w_sel'][:, g, g] = shared['w_col'][:, 0]

    meta = dict(NBLK=NBLK, NPC=NPC, TPBb=tuple(int(t) for t in TPBb),
                ET=ET, ECAP=ECAP, PW=PW)
    return meta, shared, cores


import concourse.bass as bass
import concourse.bacc as bacc
import concourse.mybir as mybir
from concourse import library_config
from concourse.tile import TileContext
from concourse.masks import make_identity
from concourse.bass_utils import run_bass_kernel_spmd

F16 = mybir.dt.float16
F32 = mybir.dt.float32
F8 = mybir.dt.float8e4
I16 = mybir.dt.int16
AX = mybir.AxisListType.X
ALU = mybir.AluOpType
AF = mybir.ActivationFunctionType


def build(meta):
    NBLK, NPC, ET, ECAP, PW = (meta[k] for k in ['NBLK', 'NPC', 'ET', 'ECAP', 'PW'])
    TPBb = meta['TPBb']
    toff = [0]
    for t in TPBb:
        toff.append(toff[-1] + t)
    nc = bacc.Bacc()

    dp = lambda n, s, d: nc.declare_dram_parameter(n, list(s), d, isOutput=False)
    x16 = dp('x16', [N, XROW], F16)
    xT_loc = dp('xT_loc', [128, NPC], F16)
    ix_x = dp('ix_x', [128, ECAP // 16], I16)
    ix_t2 = dp('ix_t2', [128, ECAP // 16], I16)
    ix_pool = dp('ix_pool', [128, GPC * PW // 16], I16)
    s8_d = dp('s8', [128, ET * 256], F8)
    enrm_d = dp('enrm', [128, ET], F32)
    mmean_d = dp('mmean', [NBLK, 128, GPC], F16)
    twin_d = dp('t_win', [32, GPC, 608], F16)
    bconv_rep = dp('bconv_rep', [GPC, 1], F32)
    wg1cat = dp('Wg1cat', [128, WAUG1], F16)
    w2ch = dp('W2chunks', [7, 128, W2AUG], F16)
    bg1ch = dp('bg1ch', [7, 128, 1], F16)
    as1f, ad1f = dp('as1f', [1, HF], F16), dp('ad1f', [1, HF], F16)
    as2f, ad2f = dp('as2f', [1, HF], F16), dp('ad2f', [1, HF], F16)
    wgcn = dp('wgcn_s', [128, F], F16)
    bgcn_col = dp('bgcn_col', [128, 1], F32)
    bgcn_row = dp('bgcn_row', [1, F], F32)
    bg2row = dp('bg2row', [1, HF], F16)
    wfg1p = dp('wfg1p', [14, 128, 128], F16)
    bfg1 = dp('bfg1', [1, 128], F32)
    wfg2p = dp('wfg2p', [2, 128, 128], F16)
    bfg2 = dp('bfg2', [1, 128], F32)
    wxtp = dp('wxtp', [5, 128, 256], F16)
    bxt = dp('bxt', [1, 256], F32)
    w1p = dp('w1p', [4, 128, 512], F16)
    b1 = dp('b1', [1, 512], F32)
    w2p = dp('w2p', [4, 128, 256], F16)
    b2 = dp('b2', [1, 256], F32)
    wop = dp('wop', [2, 128, 1], F16)
    bo_rep = dp('bo_rep', [GPC, 1], F32)
    wcol_d = dp('w_col', [32, 1], F16)
    wsel_d = dp('w_sel', [32, GPC, GPC], F16)

    out_d = nc.declare_dram_parameter('out', [GPC, 1], F32, isOutput=True)

    comb_shard = nc.dram_tensor('comb_shard', [NPC, CROW], F16)
    comb_full = nc.dram_tensor('comb_full', [8 * NPC, CROW], F16, addr_space="Shared")
    XFW = 1024
    xf_dram = nc.dram_tensor('xf_dram', [NPC, XFW], F16)

    RG = [list(range(8))]

    with TileContext(nc) as tc:
        nc.gpsimd.load_library(library_config.mlp)

        with tc.tile_pool(name="persist", bufs=1) as pp:
            w1aug_s = pp.tile([128, WAUG1], F16, tag="w1aug")
            nc.sync.dma_start(out=w1aug_s[:], in_=wg1cat[:])
            w2aug_s = pp.tile([128, 7, W2AUG], F16, tag="w2aug")
            for k in range(7):
                nc.scalar.dma_start(out=w2aug_s[:, k, :], in_=w2ch[k])
            bg1_s = pp.tile([128, 7, 1], F16, tag="bg1")
            for k in range(7):
                nc.scalar.dma_start(out=bg1_s[:, k, :], in_=bg1ch[k])
            a_s = pp.tile([128, 4, HF], F16, tag="aflat")
            for i, t in enumerate([as1f, ad1f, as2f, ad2f]):
                nc.sync.dma_start(out=a_s[:, i, :], in_=t[:].to_broadcast([128, HF]))
            xT_s = pp.tile([128, NPC], F16, tag="xT")
            nc.sync.dma_start(out=xT_s[:], in_=xT_loc[:])
            ixx_s = pp.tile([128, ECAP // 16], I16, tag="ixx")
            nc.sync.dma_start(out=ixx_s[:], in_=ix_x[:])
            ixt2_s = pp.tile([128, ECAP // 16], I16, tag="ixt2")
            nc.sync.dma_start(out=ixt2_s[:], in_=ix_t2[:])
            ixp_s = pp.tile([128, GPC * PW // 16], I16, tag="ixp")
            nc.sync.dma_start(out=ixp_s[:], in_=ix_pool[:])
            wgcn_s = pp.tile([128, F], F16, tag="wgcn")
            nc.sync.dma_start(out=wgcn_s[:], in_=wgcn[:])
            bgcnc_s = pp.tile([128, 1], F32, tag="bgcnc")
            nc.sync.dma_start(out=bgcnc_s[:], in_=bgcn_col[:])
            bgcnr_s = pp.tile([128, F], F32, tag="bgcnr")
            nc.sync.dma_start(out=bgcnr_s[:], in_=bgcn_row[:].to_broadcast([128, F]))
            bg2_s = pp.tile([128, HF], F16, tag="bg2")
            nc.sync.dma_start(out=bg2_s[:], in_=bg2row[:].to_broadcast([128, HF]))
            enrm_s = pp.tile([128, ET], F32, tag="enrm")
            nc.sync.dma_start(out=enrm_s[:], in_=enrm_d[:])
            mmean_s = pp.tile([128, NBLK, GPC], F16, tag="mmean")
            for b in range(NBLK):
                nc.sync.dma_start(out=mmean_s[:, b, :], in_=mmean_d[b])
            wcol_s = pp.tile([32, 1], F16, tag="wcol")
            nc.scalar.dma_start(out=wcol_s[:], in_=wcol_d[:])
            wsel_s = pp.tile([32, GPC, GPC], F16, tag="wsel")
            nc.scalar.dma_start(out=wsel_s[:], in_=wsel_d[:])
            bconv_s = pp.tile([GPC, 1], F32, tag="bconv")
            nc.scalar.dma_start(out=bconv_s[:], in_=bconv_rep[:])
            wfg1_s = pp.tile([128, 14, 128], F16, tag="wfg1")
            for i in range(14):
                nc.scalar.dma_start(out=wfg1_s[:, i, :], in_=wfg1p[i])
            wfg2_s = pp.tile([128, 2, 128], F16, tag="wfg2")
            for i in range(2):
                nc.scalar.dma_start(out=wfg2_s[:, i, :], in_=wfg2p[i])
            wxt_s = pp.tile([128, 5, 256], F16, tag="wxt")
            for i in range(5):
                nc.scalar.dma_start(out=wxt_s[:, i, :], in_=wxtp[i])
            w1_s = pp.tile([128, 4, 512], F16, tag="w1")
            for i in range(4):
                nc.scalar.dma_start(out=w1_s[:, i, :], in_=w1p[i])
            w2_s = pp.tile([128, 4, 256], F16, tag="w2")
            for i in range(4):
                nc.scalar.dma_start(out=w2_s[:, i, :], in_=w2p[i])
            wo_s = pp.tile([128, 2, 1], F16, tag="wo")
            for i in range(2):
                nc.scalar.dma_start(out=wo_s[:, i, :], in_=wop[i])
            bias_s = {}
            for nm, t, w in [('bfg1', bfg1, 128), ('bfg2', bfg2, 128),
                             ('bxt', bxt, 256), ('b1', b1, 512), ('b2', b2, 256)]:
                bias_s[nm] = pp.tile([GPC, w], F32, tag="bias_" + nm, name="bias_" + nm)
                nc.scalar.dma_start(out=bias_s[nm][:], in_=t[:].to_broadcast([GPC, w]))
            bo_s = pp.tile([GPC, 1], F32, tag="bo")
            nc.scalar.dma_start(out=bo_s[:], in_=bo_rep[:])

            ident_s = pp.tile([128, 128], F16, tag="ident")
            make_identity(nc, ident_s[:])
            ones_s = pp.tile([1, 128], F16, tag="ones")
            nc.vector.memset(ones_s[:], 1.0)

            bd1_s = pp.tile([128, H], F16, tag="bd1")
            adst1_s = pp.tile([128, NBLK, H], F16, tag="adst1")
            adst2_s = pp.tile([128, NBLK, H], F16, tag="adst2")
            x1loc_s = pp.tile([128, NBLK, HF], F16, tag="x1loc")
            agg1_s = pp.tile([128, NBLK, F], F16, tag="agg1")
            c2_s = pp.tile([1, W2AUG], F16, tag="c2")
            combst2 = pp.tile([128, 2, CROW], F16, tag="combst")
            nc.gpsimd.memset(combst2[:], 0.0)
            xfst2 = pp.tile([128, 2, 1024], F16, tag="xfst")
            nc.gpsimd.memset(xfst2[:], 0.0)

            # ---------------- B matrices
            with tc.tile_pool(name="bprep", bufs=2) as bp, \
                 tc.tile_pool(name="bprep_ps", bufs=2, space="PSUM") as bps:
                for i in range(2):   # B_s1 / B_d1 from Wg1
                    tmp = bp.tile([128, HF], F32, tag="btmp")
                    nc.vector.tensor_tensor(
                        out=tmp[:], in0=w1aug_s[:, 0:HF],
                        in1=a_s[:, i, :], op=ALU.mult)
                    red = bp.tile([128, H], F32, tag="bred")
                    nc.vector.tensor_reduce(
                        out=red[:], in_=tmp[:].rearrange("p (h f) -> p h f", h=H),
                        op=ALU.add, axis=AX)
                    if i == 0:
                        nc.vector.tensor_copy(out=w1aug_s[:, HF + F:WAUG1], in_=red[:])
                    else:
                        nc.vector.tensor_copy(out=bd1_s[:], in_=red[:])
                for k in range(7):
                    for i, col0 in [(2, HF), (3, HF + H)]:
                        tmp = bp.tile([128, HF], F32, tag="btmp")
                        nc.vector.tensor_tensor(
                            out=tmp[:], in0=w2aug_s[:, k, 0:HF],
                            in1=a_s[:, i, :], op=ALU.mult)
                        red = bp.tile([128, H], F32, tag="bred")
                        nc.vector.tensor_reduce(
                            out=red[:], in_=tmp[:].rearrange("p (h f) -> p h f", h=H),
                            op=ALU.add, axis=AX)
                        nc.vector.tensor_copy(out=w2aug_s[:, k, col0:col0 + H], in_=red[:])
                ps_c2 = bps.tile([1, W2AUG], F32, space="PSUM", tag="psc2")
                for k in range(7):
                    nc.tensor.matmul(out=ps_c2[:, 0:512], lhsT=bg1_s[:, k, :],
                                     rhs=w2aug_s[:, k, 0:512], start=(k == 0), stop=(k == 6))
                    nc.tensor.matmul(out=ps_c2[:, 512:W2AUG], lhsT=bg1_s[:, k, :],
                                     rhs=w2aug_s[:, k, 512:W2AUG], start=(k == 0), stop=(k == 6))
                nc.vector.tensor_copy(out=c2_s[:], in_=ps_c2[:])
                for b in range(NBLK):
                    ps_a = bps.tile([128, H], F32, space="PSUM", tag="psa")
                    nc.tensor.matmul(out=ps_a[:], lhsT=xT_s[:, 128 * b:128 * (b + 1)],
                                     rhs=bd1_s[:], start=True, stop=True)
                    nc.vector.tensor_copy(out=adst1_s[:, b, :], in_=ps_a[:])

            # ---------------- phase 1: GAT1 + GCN1
            with tc.tile_pool(name="p1", bufs=3) as p1, \
                 tc.tile_pool(name="p1g", bufs=2) as p1g, \
                 tc.tile_pool(name="p1s", bufs=2, space="PSUM") as p1s, \
                 tc.tile_pool(name="p1acc", bufs=1, space="PSUM") as p1acc:
                for b in range(NBLK):
                    TPB = TPBb[b]
                    EPB = TPB * 128
                    t0 = toff[b]
                    xgt = p1g.tile([128, 1, EPB], F16, tag="xgt")
                    nc.gpsimd.dma_gather(
                        out_ap=xgt[:], in_ap=x16[:],
                        idxs_ap=ixx_s[:, t0 * 8:(t0 + TPB) * 8],
                        num_idxs=EPB, num_idxs_reg=EPB, elem_size=XROW, transpose=True,
                        single_packet=False)
                    s8_b = p1g.tile([128, TPB, 256], F8, tag="s8b")
                    nc.sync.dma_start(out=s8_b[:], in_=s8_d[:, t0 * 256:(t0 + TPB) * 256])
                    ps_out = p1acc.tile([128, EXW], F32, space="PSUM", tag="psout", name="psout")[:]
                    for k in range(TPB):
                        s01_t = s8_b[:, k, 0:128]
                        s01t_t = s8_b[:, k, 128:256]
                        lhs = xgt[:, 0, 128 * k:128 * (k + 1)]
                        ps1 = p1s.tile([128, WAUG1], F32, space="PSUM", tag="ps1")
                        nc.tensor.matmul(out=ps1[:, 0:512], lhsT=lhs,
                                         rhs=w1aug_s[:, 0:512], start=True, stop=True)
                        nc.tensor.matmul(out=ps1[:, 512:WAUG1], lhsT=lhs,
                                         rhs=w1aug_s[:, 512:WAUG1], start=True, stop=False)
                        nc.tensor.matmul(out=ps1[:, HF + F:WAUG1],
                                         lhsT=s01t_t, rhs=adst1_s[:, b, :],
                                         start=False, stop=True)
                        exv = p1.tile([128, EXW], F16, tag="exv")
                        lr02 = p1.tile([128, H], F32, tag="lr02")
                        nc.scalar.activation(out=lr02[:], in_=ps1[:, HF + F:WAUG1],
                                             func=AF.Copy, scale=0.2)
                        lr = p1.tile([128, H], F32, tag="lr")
                        nc.vector.tensor_tensor(out=lr[:], in0=ps1[:, HF + F:WAUG1],
                                                in1=lr02[:], op=ALU.max)
                        nc.scalar.activation(out=exv[:, HF + F:EXW], in_=lr[:], func=AF.Exp)
                        nc.scalar.activation(out=exv[:, HF:HF + F], in_=ps1[:, HF:HF + F],
                                             func=AF.Copy,
                                             scale=enrm_s[:, t0 + k:t0 + k + 1])
                        nc.vector.tensor_tensor(
                            out=exv[:, 0:HF].rearrange("p (h f) -> p h f", h=H),
                            in0=ps1[:, 0:HF].rearrange("p (h f) -> p h f", h=H),
                            in1=exv[:, HF + F:EXW, None].to_broadcast([128, H, F]),
                            op=ALU.mult)
                        nc.tensor.matmul(out=ps_out[:, 0:512], lhsT=s01_t,
                                         rhs=exv[:, 0:512], start=(k == 0), stop=(k == TPB - 1))
                        nc.tensor.matmul(out=ps_out[:, 512:EXW], lhsT=s01_t,
                                         rhs=exv[:, 512:EXW], start=(k == 0), stop=(k == TPB - 1))
                    rec = p1.tile([128, H], F32, tag="rec")
                    nc.vector.reciprocal(out=rec[:], in_=ps_out[:, HF + F:EXW])
                    nc.vector.tensor_tensor(
                        out=x1loc_s[:, b, :].rearrange("p (h f) -> p h f", h=H),
                        in0=ps_out[:, 0:HF].rearrange("p (h f) -> p h f", h=H),
                        in1=rec[:, :, None].to_broadcast([128, H, F]),
                        op=ALU.mult)
                    nc.vector.tensor_copy(out=agg1_s[:, b, :], in_=ps_out[:, HF:HF + F])

            # ---------------- phase 2: comb table build + collective + conv
            with tc.tile_pool(name="p2", bufs=2) as p2:
              with tc.tile_pool(name="p2sa", bufs=2, space="PSUM") as p2s, \
                   tc.tile_pool(name="p2ta", bufs=2, space="PSUM") as p2t:
                x1t_s = p2.tile([128, 7, NPC], F16, tag="x1t", bufs=1)
                nc.gpsimd.memset(x1t_s[:], 0.0)
                for b in range(NBLK):
                    for fb in range(7):
                        c0, c1 = 128 * fb, min(128 * (fb + 1), HF)
                        psT = p2t.tile([128, 128], F16, space="PSUM", tag="psT")
                        nc.tensor.transpose(out=psT[:c1 - c0, :],
                                            in_=x1loc_s[:, b, c0:c1],
                                            identity=ident_s[:])
                        nc.vector.tensor_copy(
                            out=x1t_s[0:c1 - c0, fb, 128 * b:128 * (b + 1)],
                            in_=psT[:c1 - c0, :])
                for b in range(NBLK):
                    combst = combst2[:, b % 2, :]
                    # x2 for block b
                    psT = p2t.tile([128, 128], F16, space="PSUM", tag="psT")
                    nc.tensor.transpose(out=psT[:F, :], in_=agg1_s[:, b, :],
                                        identity=ident_s[:])
                    x2lt = p2.tile([128, 128], F16, tag="x2lt")
                    nc.vector.tensor_scalar(out=x2lt[:F, :], in0=psT[:F, :],
                                            scalar1=bgcnc_s[:F, :], scalar2=None,
                                            op0=ALU.add)
                    ps_x2 = p2s.tile([128, F], F32, space="PSUM", tag="psx2")
                    nc.tensor.matmul(out=ps_x2[:], lhsT=x2lt[:F, :], rhs=wgcn_s[:F, :],
                                     start=True, stop=True)
                    nc.vector.tensor_copy(out=combst[:, HF:HF + F], in_=ps_x2[:])
                    # T2 for block b
                    ps_t2 = p2s.tile([128, W2AUG], F32, space="PSUM", tag="pst2")
                    for k in range(7):
                        nc.tensor.matmul(out=ps_t2[:, 0:512],
                                         lhsT=x1t_s[:, k, 128 * b:128 * (b + 1)],
                                         rhs=w2aug_s[:, k, 0:512], start=(k == 0), stop=False)
                        nc.tensor.matmul(out=ps_t2[:, 512:W2AUG],
                                         lhsT=x1t_s[:, k, 128 * b:128 * (b + 1)],
                                         rhs=w2aug_s[:, k, 512:W2AUG], start=(k == 0), stop=False)
                    nc.tensor.matmul(out=ps_t2[:, 0:512], lhsT=ones_s[:],
                                     rhs=c2_s[:, 0:512], start=False, stop=True)
                    nc.tensor.matmul(out=ps_t2[:, 512:W2AUG], lhsT=ones_s[:],
                                     rhs=c2_s[:, 512:W2AUG], start=False, stop=True)
                    nc.vector.tensor_copy(out=combst[:, 0:HF], in_=ps_t2[:, 0:HF])
                    nc.vector.tensor_copy(out=combst[:, HF + F:WAUG1],
                                          in_=ps_t2[:, HF:HF + H])
                    nc.vector.tensor_copy(out=adst2_s[:, b, :], in_=ps_t2[:, HF + H:W2AUG])
                    nc.sync.dma_start(out=comb_shard[128 * b:128 * (b + 1), :],
                                      in_=combst)
                nc.gpsimd.collective_compute(
                    "AllGather", ALU.bypass, replica_groups=RG,
                    ins=[comb_shard[:]], outs=[comb_full[:]])

              with tc.tile_pool(name="p2sc", bufs=1, space="PSUM") as p2s, \
                   tc.tile_pool(name="p2tc", bufs=2, space="PSUM") as p2t:
                # conv branch overlaps the collective
                twin_s = p2.tile([32, GPC, 608], F16, tag="twin", bufs=1)
                nc.scalar.dma_start(out=twin_s[:], in_=twin_d[:])
                ps_ya = p2s.tile([GPC, 512], F32, space="PSUM", tag="psya")
                ps_yb = p2s.tile([GPC, 96], F32, space="PSUM", tag="psyb")
                for g in range(GPC):
                    nc.tensor.matmul(out=ps_ya[:], lhsT=wsel_s[:, g, :],
                                     rhs=twin_s[:, g, 0:512], start=(g == 0), stop=(g == GPC - 1))
                    nc.tensor.matmul(out=ps_yb[:], lhsT=wsel_s[:, g, :],
                                     rhs=twin_s[:, g, 512:608], start=(g == 0), stop=(g == GPC - 1))
                y_s = p2.tile([GPC, 608], F16, tag="ys")
                nc.vector.tensor_scalar(out=y_s[:, 0:512], in0=ps_ya[:],
                                        scalar1=bconv_s[:], scalar2=0.0,
                                        op0=ALU.add, op1=ALU.max)
                nc.vector.tensor_scalar(out=y_s[:, 512:608], in0=ps_yb[:],
                                        scalar1=bconv_s[:], scalar2=0.0,
                                        op0=ALU.add, op1=ALU.max)
                yt_s = pp.tile([128, 5, GPC], F16, tag="yt")
                nc.vector.memset(yt_s[:], 0.0)
                for i in range(5):
                    c0, c1 = 128 * i, min(128 * (i + 1), 608)
                    psT = p2t.tile([128, 128], F16, space="PSUM", tag="psT")
                    nc.tensor.transpose(out=psT[:c1 - c0, :GPC], in_=y_s[:, c0:c1],
                                        identity=ident_s[:GPC, :GPC])
                    nc.vector.tensor_copy(out=yt_s[0:c1 - c0, i, :], in_=psT[:c1 - c0, :GPC])
                ps_xt = p2s.tile([GPC, 256], F32, space="PSUM", tag="psxt")
                for i in range(5):
                    nc.tensor.matmul(out=ps_xt[:], lhsT=yt_s[:, i, :], rhs=wxt_s[:, i, :],
                                     start=(i == 0), stop=(i == 4))
                xt_s = p2.tile([GPC, 256], F16, tag="xts")
                nc.vector.tensor_tensor(out=xt_s[:], in0=ps_xt[:],
                                        in1=bias_s['bxt'][:], op=ALU.add)
                xtT_s = pp.tile([128, 2, GPC], F16, tag="xtT")
                for i in range(2):
                    psT = p2t.tile([128, 128], F16, space="PSUM", tag="psT")
                    nc.tensor.transpose(out=psT[:, :GPC], in_=xt_s[:, 128 * i:128 * (i + 1)],
                                        identity=ident_s[:GPC, :GPC])
                    nc.vector.tensor_copy(out=xtT_s[:, i, :], in_=psT[:, :GPC])

            # ---------------- phase 3: GAT2 + GCN2 (+ inline mean pool)
            with tc.tile_pool(name="p3", bufs=3) as p3, \
                 tc.tile_pool(name="p3g", bufs=2) as p3g, \
                 tc.tile_pool(name="p3s", bufs=2, space="PSUM") as p3s, \
                 tc.tile_pool(name="p3acc", bufs=1, space="PSUM") as p3acc, \
                 tc.tile_pool(name="p3m", bufs=1, space="PSUM") as p3m:
                ps_m = p3m.tile([GPC, HF + F], F32, space="PSUM", tag="psm", name="psm")[:]
                for b in range(NBLK):
                    TPB = TPBb[b]
                    EPB = TPB * 128
                    t0 = toff[b]
                    xfst = xfst2[:, b % 2, :]
                    v2g = p3g.tile([128, TPB, CROW], F16, tag="v2g")
                    nc.gpsimd.dma_gather(
                        out_ap=v2g[:], in_ap=comb_full[:],
                        idxs_ap=ixt2_s[:, t0 * 8:(t0 + TPB) * 8],
                        num_idxs=EPB, num_idxs_reg=EPB, elem_size=CROW,
                        single_packet=False)
                    s8_b = p3g.tile([128, TPB, 256], F8, tag="s8b3")
                    nc.sync.dma_start(out=s8_b[:], in_=s8_d[:, t0 * 256:(t0 + TPB) * 256])
                    ps_out = p3acc.tile([128, EXW], F32, space="PSUM", tag="psout3", name="psout3")[:]
                    for k in range(TPB):
                        s01_t = s8_b[:, k, 0:128]
                        s01t_t = s8_b[:, k, 128:256]
                        ps_l = p3s.tile([128, H], F32, space="PSUM", tag="psl")
                        nc.tensor.matmul(out=ps_l[:], lhsT=s01t_t,
                                         rhs=adst2_s[:, b, :], start=True, stop=False)
                        nc.tensor.matmul(out=ps_l[:], lhsT=ident_s[:],
                                         rhs=v2g[:, k, HF + F:WAUG1], start=False, stop=True)
                        exv = p3.tile([128, EXW], F16, tag="exv3")
                        lr02 = p3.tile([128, H], F32, tag="lr023")
                        nc.scalar.activation(out=lr02[:], in_=ps_l[:], func=AF.Copy, scale=0.2)
                        lr = p3.tile([128, H], F32, tag="lr3")
                        nc.vector.tensor_tensor(out=lr[:], in0=ps_l[:], in1=lr02[:], op=ALU.max)
                        nc.scalar.activation(out=exv[:, HF + F:EXW], in_=lr[:], func=AF.Exp)
                        nc.scalar.activation(out=exv[:, HF:HF + F], in_=v2g[:, k, HF:HF + F],
                                             func=AF.Copy,
                                             scale=enrm_s[:, t0 + k:t0 + k + 1])
                        nc.vector.tensor_tensor(
                            out=exv[:, 0:HF].rearrange("p (h f) -> p h f", h=H),
                            in0=v2g[:, k, 0:HF].rearrange("p (h f) -> p h f", h=H),
                            in1=exv[:, HF + F:EXW, None].to_broadcast([128, H, F]),
                            op=ALU.mult)
                        nc.tensor.matmul(out=ps_out[:, 0:512], lhsT=s01_t,
                                         rhs=exv[:, 0:512], start=(k == 0), stop=(k == TPB - 1))
                        nc.tensor.matmul(out=ps_out[:, 512:EXW], lhsT=s01_t,
                                         rhs=exv[:, 512:EXW], start=(k == 0), stop=(k == TPB - 1))
                    rec = p3.tile([128, H], F32, tag="rec3")
                    nc.vector.reciprocal(out=rec[:], in_=ps_out[:, HF + F:EXW])
                    u_s = p3.tile([128, HF], F16, tag="us")
                    nc.vector.tensor_tensor(
                        out=u_s[:].rearrange("p (h f) -> p h f", h=H),
                        in0=ps_out[:, 0:HF].rearrange("p (h f) -> p h f", h=H),
                        in1=rec[:, :, None].to_broadcast([128, H, F]),
                        op=ALU.mult)
                    v_s = p3.tile([128, HF], F16, tag="vs")
                    nc.vector.tensor_tensor(out=v_s[:], in0=u_s[:], in1=bg2_s[:],
                                            op=ALU.add)
                    nc.scalar.activation(out=xfst[:, 0:HF], in_=v_s[:], func=AF.Relu)
                    g2f = p3.tile([128, F], F32, tag="g2f")
                    nc.vector.tensor_tensor(out=g2f[:], in0=ps_out[:, HF:HF + F],
                                            in1=bgcnr_s[:], op=ALU.add)
                    nc.scalar.activation(out=xfst[:, HF:HF + F], in_=g2f[:], func=AF.Relu)
                    nc.scalar.activation(out=xfst[:, 896:896 + F], in_=g2f[:], func=AF.Relu)
                    nc.sync.dma_start(out=xf_dram[128 * b:128 * (b + 1), :], in_=xfst[:])
                    nc.tensor.matmul(out=ps_m[:, 0:512], lhsT=mmean_s[:, b, :],
                                     rhs=xfst[:, 0:512], start=(b == 0),
                                     stop=(b == NBLK - 1))
                    nc.tensor.matmul(out=ps_m[:, 512:HF + F], lhsT=mmean_s[:, b, :],
                                     rhs=xfst[:, 512:HF + F], start=(b == 0),
                                     stop=(b == NBLK - 1))
                mean_s = pp.tile([GPC, HF + F], F16, tag="means")
                nc.vector.tensor_copy(out=mean_s[:], in_=ps_m[:])

            # ---------------- phase 4: max pool + head
            with tc.tile_pool(name="p4", bufs=2) as p4:
              with tc.tile_pool(name="p4s", bufs=1, space="PSUM") as p4s:
                gmax1T = pp.tile([128, 7, GPC], F16, tag="gmax1T")
                nc.vector.memset(gmax1T[:], 0.0)
                gmax2T = pp.tile([128, 1, GPC], F16, tag="gmax2T")
                nc.vector.memset(gmax2T[:], 0.0)
                CH = GPC // 2
                for h in range(2):
                    slab = p4.tile([128, 7, CH * PW], F16, tag="slab")
                    nc.gpsimd.dma_gather(
                        out_ap=slab[:], in_ap=xf_dram[:, 0:CROW],
                        idxs_ap=ixp_s[:, h * (CH * PW // 16):(h + 1) * (CH * PW // 16)],
                        num_idxs=CH * PW, num_idxs_reg=CH * PW, elem_size=CROW,
                        elem_step=1024, transpose=True, single_packet=False)
                    slab2 = p4.tile([128, 1, CH * PW], F16, tag="slab2")
                    nc.gpsimd.dma_gather(
                        out_ap=slab2[:], in_ap=xf_dram[:, 896:1024],
                        idxs_ap=ixp_s[:, h * (CH * PW // 16):(h + 1) * (CH * PW // 16)],
                        num_idxs=CH * PW, num_idxs_reg=CH * PW, elem_size=XROW,
                        elem_step=1024, transpose=True, single_packet=False)
                    for g in range(CH):
                        for j in range(7):
                            nc.vector.tensor_reduce(
                                out=gmax1T[:, j, h * CH + g:h * CH + g + 1],
                                in_=slab[:, j, g * PW:(g + 1) * PW],
                                op=ALU.max, axis=AX)
                        nc.vector.tensor_reduce(
                            out=gmax2T[:, 0, h * CH + g:h * CH + g + 1],
                            in_=slab2[:, 0, g * PW:(g + 1) * PW],
                            op=ALU.max, axis=AX)
              with tc.tile_pool(name="p4sh", bufs=1, space="PSUM") as p4s:
                gmean1T = pp.tile([128, 7, GPC], F16, tag="gmean1T")
                nc.vector.memset(gmean1T[:], 0.0)
                gmean2T = pp.tile([128, 1, GPC], F16, tag="gmean2T")
                nc.vector.memset(gmean2T[:], 0.0)
                for i in range(7):
                    c0, c1 = 128 * i, min(128 * (i + 1), HF)
                    psT = p4s.tile([128, 128], F16, space="PSUM", tag="psT4", bufs=2)
                    nc.tensor.transpose(out=psT[:c1 - c0, :GPC], in_=mean_s[:, c0:c1],
                                        identity=ident_s[:GPC, :GPC])
                    nc.vector.tensor_copy(out=gmean1T[0:c1 - c0, i, :], in_=psT[:c1 - c0, :GPC])
                psT = p4s.tile([128, 128], F16, space="PSUM", tag="psT4", bufs=2)
                nc.tensor.transpose(out=psT[:F, :GPC], in_=mean_s[:, HF:HF + F],
                                    identity=ident_s[:GPC, :GPC])
                nc.vector.tensor_copy(out=gmean2T[0:F, 0, :], in_=psT[:F, :GPC])

                def head_mm(ps, chunks, rhs_tile, nw):
                    n = len(chunks)
                    for i, ch in enumerate(chunks):
                        nc.tensor.matmul(out=ps[:], lhsT=ch, rhs=rhs_tile[:, i, :nw],
                                         start=(i == 0), stop=(i == n - 1))

                def bias_relu_T(ps, bias_ap, w, relu, nT, tagb):
                    zs = p4.tile([GPC, w], F16, tag="z" + tagb)
                    nc.vector.tensor_tensor(out=zs[:], in0=ps[:], in1=bias_ap, op=ALU.add)
                    if relu:
                        nc.vector.tensor_scalar(out=zs[:], in0=zs[:], scalar1=0.0,
                                                scalar2=None, op0=ALU.max)
                    zT = pp.tile([128, nT, GPC], F16, tag="zT" + tagb)
                    for i in range(nT):
                        psT2 = p4s.tile([128, 128], F16, space="PSUM", tag="psT4", bufs=2)
                        nc.tensor.transpose(out=psT2[:, :GPC], in_=zs[:, 128 * i:128 * (i + 1)],
                                            identity=ident_s[:GPC, :GPC])
                        nc.vector.tensor_copy(out=zT[:, i, :], in_=psT2[:, :GPC])
                    return zT

                ps_z1 = p4s.tile([GPC, 128], F32, space="PSUM", tag="psz1")
                head_mm(ps_z1, [gmax1T[:, j, :] for j in range(7)]
                        + [gmean1T[:, j, :] for j in range(7)], wfg1_s, 128)
                z1T = bias_relu_T(ps_z1, bias_s['bfg1'][:], 128, True, 1, "1")
                ps_z2 = p4s.tile([GPC, 128], F32, space="PSUM", tag="psz2")
                head_mm(ps_z2, [gmax2T[:, 0, :], gmean2T[:, 0, :]], wfg2_s, 128)
                z2T = bias_relu_T(ps_z2, bias_s['bfg2'][:], 128, True, 1, "2")
                ps_h1 = p4s.tile([GPC, 512], F32, space="PSUM", tag="psh1")
                head_mm(ps_h1, [z1T[:, 0, :], z2T[:, 0, :], xtT_s[:, 0, :], xtT_s[:, 1, :]],
                        w1_s, 512)
                h1T = bias_relu_T(ps_h1, bias_s['b1'][:], 512, True, 4, "h1")
                ps_h2 = p4s.tile([GPC, 256], F32, space="PSUM", tag="psh2")
                head_mm(ps_h2, [h1T[:, i, :] for i in range(4)], w2_s, 256)
                h2T = bias_relu_T(ps_h2, bias_s['b2'][:], 256, True, 2, "h2")
                ps_o = p4s.tile([GPC, 1], F32, space="PSUM", tag="pso")
                head_mm(ps_o, [h2T[:, i, :] for i in range(2)], wo_s, 1)
                o_s = p4.tile([GPC, 1], F32, tag="os")
                nc.vector.tensor_scalar(out=o_s[:], in0=ps_o[:], scalar1=bo_s[:],
                                        scalar2=None, op0=ALU.add)
                nc.sync.dma_start(out=out_d[:], in_=o_s[:])

    nc.compile()
    return nc


def build_in_maps(nc, shared, cores):
    declared = set()
    import concourse.mybir as _mb
    for alloc in nc.m.functions[0].allocations:
        if isinstance(alloc, _mb.MemoryLocationSet) and alloc.kind == "ExternalInput":
            declared.add(alloc.memorylocations[0].name)
    in_maps = []
    for c in range(8):
        m = dict(shared)
        m.update(cores[c])
        in_maps.append({k: np.ascontiguousarray(v) for k, v in m.items()
                        if k in declared})
    return in_maps


_CACHE = {}


def run_device(inputs):
    meta, shared, cores = prep(**inputs)
    key = (meta['NBLK'], meta['TPBb'], meta['PW'])
    if key not in _CACHE:
        _CACHE[key] = build(meta)
    nc = _CACHE[key]
    in_maps = build_in_maps(nc, shared, cores)
    res = run_bass_kernel_spmd(nc, in_maps, core_ids=list(range(8)))
    out = np.concatenate([res.results[c]['out'] for c in range(8)], axis=0)
    return out.astype(np.float32)


def kernel(**inputs):
    return run_device(inputs)


# revision 13
# speedup vs baseline: 1.3863x; 1.2475x over previous
"""Trainium2 Bass kernel for nn_GAT_GCN (gnn_message_passing), 8 NeuronCores.

v2 strategy (from v1 baseline at 1075us):
 - Dst-node sharding, graph-aligned (16 graphs/core). Within a core, nodes
   are BIN-PACKED into 128-node blocks balancing per-block edge counts, so
   every block needs the same tile count (TPB~9 vs v1's 10) -> 153 tiles
   instead of 170.
 - One merged exchange row per node: [h2(780) | x2(78) | asrc2(10) | pad]
   = 896 f16 = 1792B. One AllGather (31.2MB) and ONE phase-3 gather per
   edge (v1 had two gathers + 1024-col rows).
 - Scatter matrices s01/s01t in fp8 (exact 0/1); GCN's norm is applied via
   an extra broadcast column in the exp multiply (ex_ext[:,10]=norm_e), so
   the separate snrm matrix is gone.
 - Per-edge scale multiply (exv) runs in DVE 2x mode: PSUM->SBUF fp16 copy
   on ACT for most tiles, direct-PSUM for the rest (engine balance).
 - Leakyrelu max on gpsimd; denominators folded into the single scatter
   matmul pair (rhs = [exv(858) | ex(10)]).
 - Mean-pool accumulated inline in phase 3 (PE matmul); max-pool gathers
   read one merged x1f|x2f table; reduces split DVE/gpsimd.
"""
import sys
sys.path.insert(0, '/opt/trn_rl_repo')
import numpy as np
import ml_dtypes

N, E, G, F, H = 16384, 131072, 128, 78, 10
NCORE, GPC = 8, 16
HF = H * F                  # 780
WAUG1 = HF + F + H          # 868 = h1(780) | hgcn(78) | asrc(10)
W2AUG = HF + 2 * H          # 800 = h2 | asrc2 | adst2
CROW = 896                  # comb row: h2(780) | x2(78) | asrc2(10) | pad
XROW = 128                  # x table row, fp16 (256B)
EXW = WAUG1                 # 868 = exv(858) | ex(10) scatter rhs width


def _wrap16(v):
    v = np.asarray(v, np.int16)
    assert len(v) % 16 == 0
    m = v.reshape(-1, 16).T
    return np.tile(m, (8, 1)).copy()


def _f16(a):
    return np.ascontiguousarray(np.asarray(a, np.float32)).astype(np.float16)


def _f8(a):
    return np.ascontiguousarray(np.asarray(a, np.float32)).astype(ml_dtypes.float8_e4m3)


def prep(x, edge_index, batch, target, Wg1, as1, ad1, bg1, Wg2, as2, ad2, bg2,
         Wgcn, bgcn, Wfg1, bfg1, Wfg2, bfg2, wconv, bconv, Wxt, bxt,
         W1, b1, W2, b2, Wo, bo):
    x = np.asarray(x, np.float32)
    ei = np.asarray(edge_index, np.int64)
    batch = np.asarray(batch, np.int64)
    target = np.asarray(target, np.float32)

    loops = np.arange(N, dtype=np.int64)
    src = np.concatenate([ei[0], loops])
    dst = np.concatenate([ei[1], loops])

    counts = np.bincount(batch, minlength=G)
    node_off = np.concatenate([[0], np.cumsum(counts)])
    n_lo = node_off[np.arange(NCORE) * GPC]
    n_hi = node_off[(np.arange(NCORE) + 1) * GPC]

    deg = np.bincount(dst, minlength=N).astype(np.float64)
    dinv = 1.0 / np.sqrt(deg)
    norm = (dinv[src] * dinv[dst]).astype(np.float32)

    Lmax = int((n_hi - n_lo).max())
    NBLK = (Lmax + 127) // 128
    NPC = NBLK * 128
    assert NCORE * NPC < 32768

    # ---- per-core bin packing of nodes into NBLK blocks of <=128 nodes,
    # balancing per-block edge (degree) sums.
    blk_of = np.zeros(N, np.int64)     # block index of node (within its core)
    slot_of = np.zeros(N, np.int64)    # slot within block
    tiles_cb = np.zeros((NCORE, NBLK), np.int64)
    for c in range(NCORE):
        ids = np.arange(n_lo[c], n_hi[c])
        degs = deg[ids]
        order = np.argsort(-degs, kind='stable')
        bins_e = np.zeros(NBLK)
        bins_n = np.zeros(NBLK, np.int64)
        for i in order:
            cand = np.where(bins_n < 128, bins_e, np.inf)
            bsel = int(np.argmin(cand))
            nid = ids[i]
            blk_of[nid] = bsel
            slot_of[nid] = bins_n[bsel]
            bins_e[bsel] += degs[i]
            bins_n[bsel] += 1
        tiles_cb[c] = (bins_e.astype(np.int64) + 127) // 128
    TPBb = tiles_cb.max(axis=0)        # per-block tile count (same all cores)
    toff = np.concatenate([[0], np.cumsum(TPBb)])
    ET = int(toff[-1])
    ECAP = ET * 128
    node_owner = np.searchsorted(n_hi - 1, np.arange(N), side='left')
    node_owner = np.minimum(node_owner, NCORE - 1)
    local_id = blk_of * 128 + slot_of                    # 0..NPC-1
    pad_gid = node_owner * NPC + local_id

    # edges sorted by (core, block)
    ecore = node_owner[dst]
    eblk = blk_of[dst]
    ekey = ecore * NBLK + eblk
    order = np.argsort(ekey, kind='stable')
    srcs, dsts = src[order], dst[order]
    norms = norm[order]
    ekey_s = ekey[order]

    PW = int(np.ceil(counts.max() / 16) * 16)

    # (computed below in weight prep, hoisted here for the core loop)
    P = (np.arange(F)[:, None] + F * np.arange(H)[None, :]).reshape(-1)
    Wg1_h = np.asarray(Wg1, np.float32)[:, P]
    ad1a_h = np.asarray(ad1, np.float32)
    B_d1_h = np.einsum('xfh,hf->xh', Wg1_h.reshape(F, F, H), ad1a_h)
    adst1_all = (x @ B_d1_h).astype(np.float16)

    cores = []
    for c in range(NCORE):
        esrc = np.zeros(ECAP, np.int64)
        s01 = np.zeros((ET, 128, 128), ml_dtypes.float8_e4m3)
        s01t = np.zeros((ET, 128, 128), ml_dtypes.float8_e4m3)
        enrm = np.zeros((ET, 128), np.float16)    # [tile, edge-slot] norm
        for b in range(NBLK):
            lo = np.searchsorted(ekey_s, c * NBLK + b)
            hi = np.searchsorted(ekey_s, c * NBLK + b, side='right')
            ne = hi - lo
            t0 = toff[b]
            if ne > 0:
                j = np.arange(ne)
                t_loc = j // 128
                e_loc = j % 128
                ld = slot_of[dsts[lo:hi]]
                gslot = (t0 + t_loc) * 128 + e_loc
                esrc[gslot] = srcs[lo:hi]
                s01[t0 + t_loc, e_loc, ld] = 1.0
                s01t[t0 + t_loc, ld, e_loc] = 1.0
                enrm[t0 + t_loc, e_loc] = norms[lo:hi].astype(np.float16)
            # pad dst slots (no nodes) get a fake denominator entry
            nnode = int(((node_owner == c) & (blk_of == b)).sum()) if False else None
        # count nodes per (c, b) to set fake denominators on empty slots
        nb = np.zeros(NBLK, np.int64)
        sel = np.arange(n_lo[c], n_hi[c])
        for b in range(NBLK):
            nb[b] = int((blk_of[sel] == b).sum())
        for b in range(NBLK):
            if nb[b] < 128:
                s01[toff[b], 0, nb[b]:] = 1.0

        # pooling indices (local node ids into xf table)
        pool_idx = np.zeros(GPC * PW, np.int64)
        for g in range(GPC):
            gg = c * GPC + g
            ids = np.arange(node_off[gg], node_off[gg + 1])
            lid = local_id[ids]
            cnt = len(ids)
            pool_idx[g * PW:g * PW + cnt] = lid
            pool_idx[g * PW + cnt:(g + 1) * PW] = lid[0]
        mmean = np.zeros((NBLK, 128, GPC), np.float16)
        for g in range(GPC):
            gg = c * GPC + g
            ids = np.arange(node_off[gg], node_off[gg + 1])
            mmean[blk_of[ids], slot_of[ids], g] = np.float16(1.0 / len(ids))

        t_win = np.zeros((32, GPC, 608), np.float16)
        tg = target[c * GPC:(c + 1) * GPC, 0, :]
        for k in range(32):
            t_win[k, :, :594] = tg[:, k:k + 594].astype(np.float16)

        xT = np.zeros((128, NPC), np.float16)
        ids = np.arange(n_lo[c], n_hi[c])
        xT[:F, local_id[ids]] = x[ids, :].T.astype(np.float16)

        # combined fp8 s-matrix stream: [partition, tile*(s01|s01t)]
        s8 = np.zeros((128, ET, 256), ml_dtypes.float8_e4m3)
        s8[:, :, 0:128] = s01.transpose(1, 0, 2)
        s8[:, :, 128:256] = s01t.transpose(1, 0, 2)
        s8 = s8.reshape(128, ET * 256)
        ad1t = np.zeros((128, NBLK * H), np.float16)
        idsc = np.arange(n_lo[c], n_hi[c])
        for b in range(NBLK):
            selb = idsc[blk_of[idsc] == b]
            ad1t[slot_of[selb], b * H:(b + 1) * H] = adst1_all[selb]
        cores.append(dict(
            adst1=ad1t,
            ix_x=_wrap16(esrc),
            ix_t2=_wrap16(pad_gid[esrc]),
            ix_pool=_wrap16(pool_idx),
            s8=s8, enrm=np.ascontiguousarray(enrm.T.astype(np.float32)),
            mmean=mmean, t_win=t_win, xT_loc=xT,
            bconv_rep=np.full((GPC, 1), float(bconv[0]), np.float32),
        ))

    x16 = np.zeros((N, XROW), np.float16)
    x16[:, :F] = x.astype(np.float16)

    # f-major permutation for the head dim: new index f*H+h <- old h*F+f
    P = (np.arange(F)[:, None] + F * np.arange(H)[None, :]).reshape(-1)  # P[f*H+h]=h*F+f
    Wg1 = np.asarray(Wg1, np.float32)[:, P]
    Wg2p = np.asarray(Wg2, np.float32)[P][:, P]          # rows and cols f-major
    as1a = np.asarray(as1, np.float32)
    ad1a = np.asarray(ad1, np.float32)
    as2a = np.asarray(as2, np.float32)
    ad2a = np.asarray(ad2, np.float32)
    bg1p = np.asarray(bg1, np.float32).reshape(H, F).T.reshape(-1)       # f-major
    bg2p = np.asarray(bg2, np.float32).reshape(H, F).T.reshape(-1)
    # B vectors (host): B[x_row, h] = sum_f W[x_row, fm(f,h)] * a[h, f]
    Wg1_r = Wg1.reshape(F, F, H)                          # [xf, f, h]
    B_s1 = np.einsum('xfh,hf->xh', Wg1_r, as1a)
    B_d1 = np.einsum('xfh,hf->xh', Wg1_r, ad1a)
    Wg2_r = Wg2p.reshape(HF, F, H)
    B_s2 = np.einsum('xfh,hf->xh', Wg2_r, as2a)           # [780(fm), 10]
    B_d2 = np.einsum('xfh,hf->xh', Wg2_r, ad2a)
    # c2 = bg1 @ W2aug  (x1 bias folded into T2)
    c2 = np.concatenate([bg1p @ Wg2p, bg1p @ B_s2, bg1p @ B_d2]).reshape(1, W2AUG)

    Wg1cat = np.zeros((128, WAUG1), np.float16)
    Wg1cat[:F, :HF] = _f16(Wg1)
    Wg1cat[:F, HF:HF + F] = _f16(Wgcn)
    Wg1cat[:F, HF + F:WAUG1] = _f16(B_s1)
    W2chunks = np.zeros((7, 128, W2AUG), np.float16)
    W2a = np.concatenate([Wg2p, B_s2, B_d2], axis=1)      # [780, 800]
    for k in range(7):
        r0, r1 = 128 * k, min(128 * (k + 1), HF)
        W2chunks[k, :r1 - r0, :] = _f16(W2a[r0:r1, :])

    def pack_rows(Wm, splits, ncol):
        out = np.zeros((len(splits), 128, ncol), np.float16)
        for i, (r0, r1) in enumerate(splits):
            out[i, :r1 - r0, :] = _f16(Wm[r0:r1, :])
        return out

    sp7 = [(128 * i, min(128 * (i + 1), HF)) for i in range(7)]
    Wfg1a = np.asarray(Wfg1, np.float32)
    Wfg1_perm = np.concatenate([Wfg1a[:HF][P], Wfg1a[HF:][P]], axis=0)
    wfg1p = np.concatenate([pack_rows(Wfg1_perm[:HF], sp7, 128),
                            pack_rows(Wfg1_perm[HF:], sp7, 128)], axis=0)
    wfg2p = pack_rows(Wfg2, [(0, F), (F, 2 * F)], 128)
    wxtp = pack_rows(Wxt, [(128 * i, min(128 * (i + 1), 594)) for i in range(5)], 256)
    w1p = pack_rows(W1, [(128 * i, 128 * (i + 1)) for i in range(4)], 512)
    w2p = pack_rows(W2, [(128 * i, 128 * (i + 1)) for i in range(4)], 256)
    wop = pack_rows(Wo, [(0, 128), (128, 256)], 1)

    wgcn_s = np.zeros((128, F), np.float16)
    wgcn_s[:F] = _f16(Wgcn)
    bgcn_col = np.zeros((128, 1), np.float32)
    bgcn_col[:F, 0] = np.asarray(bgcn, np.float32)

    shared = dict(
        x16=x16, Wg1cat=Wg1cat, W2chunks=W2chunks,
        c2row=_f16(c2),
        wgcn_s=wgcn_s, bgcn_col=bgcn_col,
        bgcn_row=np.asarray(bgcn, np.float32).reshape(1, F),
        bg2row=_f16(bg2p).reshape(1, HF),
        wfg1p=wfg1p, bfg1=np.asarray(bfg1, np.float32).reshape(1, 128),
        wfg2p=wfg2p, bfg2=np.asarray(bfg2, np.float32).reshape(1, 128),
        wxtp=wxtp, bxt=np.asarray(bxt, np.float32).reshape(1, 256),
        w1p=w1p, b1=np.asarray(b1, np.float32).reshape(1, 512),
        w2p=w2p, b2=np.asarray(b2, np.float32).reshape(1, 256),
        wop=wop, bo_rep=np.full((GPC, 1), float(np.asarray(bo).reshape(-1)[0]), np.float32),
        w_col=np.zeros((32, 1), np.float16),
        w_sel=np.zeros((32, GPC, GPC), np.float16),
    )
    shared['w_col'][:, 0] = _f16(np.asarray(wconv).reshape(-1))
    for g in range(GPC):
        shared['w_sel'][:, g, g] = shared['w_col'][:, 0]

    meta = dict(NBLK=NBLK, NPC=NPC, TPBb=tuple(int(t) for t in TPBb),
                ET=ET, ECAP=ECAP, PW=PW)
    return meta, shared, cores


import concourse.bass as bass
import concourse.bacc as bacc
import concourse.mybir as mybir
from concourse import library_config
from concourse.tile import TileContext
from concourse.masks import make_identity
from concourse.bass_utils import run_bass_kernel_spmd

F16 = mybir.dt.float16
F32 = mybir.dt.float32
F8 = mybir.dt.float8e4
I16 = mybir.dt.int16
AX = mybir.AxisListType.X
ALU = mybir.AluOpType
AF = mybir.ActivationFunctionType


def build(meta):
    NBLK, NPC, ET, ECAP, PW = (meta[k] for k in ['NBLK', 'NPC', 'ET', 'ECAP', 'PW'])
    TPBb = meta['TPBb']
    toff = [0]
    for t in TPBb:
        toff.append(toff[-1] + t)
    nc = bacc.Bacc()

    dp = lambda n, s, d: nc.declare_dram_parameter(n, list(s), d, isOutput=False)
    x16 = dp('x16', [N, XROW], F16)
    ix_x = dp('ix_x', [128, ECAP // 16], I16)
    ix_t2 = dp('ix_t2', [128, ECAP // 16], I16)
    ix_pool = dp('ix_pool', [128, GPC * PW // 16], I16)
    s8_d = dp('s8', [128, ET * 256], F8)
    enrm_d = dp('enrm', [128, ET], F32)
    adst1_d = dp('adst1', [128, NBLK * H], F16)
    c2_d = dp('c2row', [1, W2AUG], F16)
    mmean_d = dp('mmean', [NBLK, 128, GPC], F16)
    twin_d = dp('t_win', [32, GPC, 608], F16)
    bconv_rep = dp('bconv_rep', [GPC, 1], F32)
    wg1cat = dp('Wg1cat', [128, WAUG1], F16)
    w2ch = dp('W2chunks', [7, 128, W2AUG], F16)
    wgcn = dp('wgcn_s', [128, F], F16)
    bgcn_col = dp('bgcn_col', [128, 1], F32)
    bgcn_row = dp('bgcn_row', [1, F], F32)
    bg2row = dp('bg2row', [1, HF], F16)
    wfg1p = dp('wfg1p', [14, 128, 128], F16)
    bfg1 = dp('bfg1', [1, 128], F32)
    wfg2p = dp('wfg2p', [2, 128, 128], F16)
    bfg2 = dp('bfg2', [1, 128], F32)
    wxtp = dp('wxtp', [5, 128, 256], F16)
    bxt = dp('bxt', [1, 256], F32)
    w1p = dp('w1p', [4, 128, 512], F16)
    b1 = dp('b1', [1, 512], F32)
    w2p = dp('w2p', [4, 128, 256], F16)
    b2 = dp('b2', [1, 256], F32)
    wop = dp('wop', [2, 128, 1], F16)
    bo_rep = dp('bo_rep', [GPC, 1], F32)
    wcol_d = dp('w_col', [32, 1], F16)
    wsel_d = dp('w_sel', [32, GPC, GPC], F16)

    out_d = nc.declare_dram_parameter('out', [GPC, 1], F32, isOutput=True)

    comb_shard = nc.dram_tensor('comb_shard', [NPC, CROW], F16)
    comb_full = nc.dram_tensor('comb_full', [8 * NPC, CROW], F16, addr_space="Shared")
    XFW = 1024
    xf_dram = nc.dram_tensor('xf_dram', [NPC, XFW], F16)

    RG = [list(range(8))]

    with TileContext(nc) as tc:
        nc.gpsimd.load_library(library_config.mlp)

        with tc.tile_pool(name="persist", bufs=1) as pp:
            w1aug_s = pp.tile([128, WAUG1], F16, tag="w1aug")
            nc.sync.dma_start(out=w1aug_s[:], in_=wg1cat[:])
            w2aug_s = pp.tile([128, 7, W2AUG], F16, tag="w2aug")
            for k in range(7):
                nc.scalar.dma_start(out=w2aug_s[:, k, :], in_=w2ch[k])
            ixx_s = pp.tile([128, ECAP // 16], I16, tag="ixx")
            nc.sync.dma_start(out=ixx_s[:], in_=ix_x[:])
            ixt2_s = pp.tile([128, ECAP // 16], I16, tag="ixt2")
            nc.sync.dma_start(out=ixt2_s[:], in_=ix_t2[:])
            ixp_s = pp.tile([128, GPC * PW // 16], I16, tag="ixp")
            nc.sync.dma_start(out=ixp_s[:], in_=ix_pool[:])
            wgcn_s = pp.tile([128, F], F16, tag="wgcn")
            nc.sync.dma_start(out=wgcn_s[:], in_=wgcn[:])
            bgcnc_s = pp.tile([128, 1], F32, tag="bgcnc")
            nc.sync.dma_start(out=bgcnc_s[:], in_=bgcn_col[:])
            bgcnr_s = pp.tile([128, F], F32, tag="bgcnr")
            nc.sync.dma_start(out=bgcnr_s[:], in_=bgcn_row[:].to_broadcast([128, F]))
            bg2_s = pp.tile([128, HF], F16, tag="bg2")
            nc.sync.dma_start(out=bg2_s[:], in_=bg2row[:].to_broadcast([128, HF]))
            enrm_s = pp.tile([128, ET], F32, tag="enrm")
            nc.sync.dma_start(out=enrm_s[:], in_=enrm_d[:])
            mmean_s = pp.tile([128, NBLK, GPC], F16, tag="mmean")
            for b in range(NBLK):
                nc.sync.dma_start(out=mmean_s[:, b, :], in_=mmean_d[b])
            wcol_s = pp.tile([32, 1], F16, tag="wcol")
            nc.scalar.dma_start(out=wcol_s[:], in_=wcol_d[:])
            wsel_s = pp.tile([32, GPC, GPC], F16, tag="wsel")
            nc.scalar.dma_start(out=wsel_s[:], in_=wsel_d[:])
            bconv_s = pp.tile([GPC, 1], F32, tag="bconv")
            nc.scalar.dma_start(out=bconv_s[:], in_=bconv_rep[:])
            wfg1_s = pp.tile([128, 14, 128], F16, tag="wfg1")
            for i in range(14):
                nc.scalar.dma_start(out=wfg1_s[:, i, :], in_=wfg1p[i])
            wfg2_s = pp.tile([128, 2, 128], F16, tag="wfg2")
            for i in range(2):
                nc.scalar.dma_start(out=wfg2_s[:, i, :], in_=wfg2p[i])
            wxt_s = pp.tile([128, 5, 256], F16, tag="wxt")
            for i in range(5):
                nc.scalar.dma_start(out=wxt_s[:, i, :], in_=wxtp[i])
            w1_s = pp.tile([128, 4, 512], F16, tag="w1")
            for i in range(4):
                nc.scalar.dma_start(out=w1_s[:, i, :], in_=w1p[i])
            w2_s = pp.tile([128, 4, 256], F16, tag="w2")
            for i in range(4):
                nc.scalar.dma_start(out=w2_s[:, i, :], in_=w2p[i])
            wo_s = pp.tile([128, 2, 1], F16, tag="wo")
            for i in range(2):
                nc.scalar.dma_start(out=wo_s[:, i, :], in_=wop[i])
            bias_s = {}
            for nm, t, w in [('bfg1', bfg1, 128), ('bfg2', bfg2, 128),
                             ('bxt', bxt, 256), ('b1', b1, 512), ('b2', b2, 256)]:
                bias_s[nm] = pp.tile([GPC, w], F32, tag="bias_" + nm, name="bias_" + nm)
                nc.scalar.dma_start(out=bias_s[nm][:], in_=t[:].to_broadcast([GPC, w]))
            bo_s = pp.tile([GPC, 1], F32, tag="bo")
            nc.scalar.dma_start(out=bo_s[:], in_=bo_rep[:])

            ident_s = pp.tile([128, 128], F16, tag="ident")
            make_identity(nc, ident_s[:])
            ones_s = pp.tile([1, 128], F16, tag="ones")
            nc.vector.memset(ones_s[:], 1.0)

            adst1_s = pp.tile([128, NBLK, H], F16, tag="adst1")
            nc.sync.dma_start(out=adst1_s[:].rearrange("p b h -> p (b h)"), in_=adst1_d[:])
            adst2_s = pp.tile([128, NBLK, H], F16, tag="adst2")
            x1loc_s = pp.tile([128, NBLK, HF], F16, tag="x1loc")
            agg1_s = pp.tile([128, NBLK, F], F16, tag="agg1")
            c2_s = pp.tile([1, W2AUG], F16, tag="c2")
            nc.sync.dma_start(out=c2_s[:], in_=c2_d[:])
            combst2 = pp.tile([128, 2, CROW], F16, tag="combst")
            nc.gpsimd.memset(combst2[:], 0.0)
            xfst2 = pp.tile([128, 2, 1024], F16, tag="xfst")
            nc.gpsimd.memset(xfst2[:], 0.0)

            # ---------------- phase 1: GAT1 + GCN1
            with tc.tile_pool(name="p1", bufs=3) as p1, \
                 tc.tile_pool(name="p1g", bufs=2) as p1g, \
                 tc.tile_pool(name="p1s", bufs=3, space="PSUM") as p1s, \
                 tc.tile_pool(name="p1acc", bufs=1, space="PSUM") as p1acc:
                for b in range(NBLK):
                    TPB = TPBb[b]
                    EPB = TPB * 128
                    t0 = toff[b]
                    xgt = p1g.tile([128, 1, EPB], F16, tag="xgt")
                    nc.gpsimd.dma_gather(
                        out_ap=xgt[:], in_ap=x16[:],
                        idxs_ap=ixx_s[:, t0 * 8:(t0 + TPB) * 8],
                        num_idxs=EPB, num_idxs_reg=EPB, elem_size=XROW, transpose=True,
                        single_packet=False)
                    s8_b = p1g.tile([128, TPB, 256], F8, tag="s8b")
                    nc.sync.dma_start(out=s8_b[:], in_=s8_d[:, t0 * 256:(t0 + TPB) * 256])
                    ps_out = p1acc.tile([128, EXW], F32, space="PSUM", tag="psout", name="psout")[:]
                    for k in range(TPB):
                        s01_t = s8_b[:, k, 0:128]
                        s01t_t = s8_b[:, k, 128:256]
                        lhs = xgt[:, 0, 128 * k:128 * (k + 1)]
                        ps1 = p1s.tile([128, WAUG1], F32, space="PSUM", tag="ps1")
                        nc.tensor.matmul(out=ps1[:, 0:512], lhsT=lhs,
                                         rhs=w1aug_s[:, 0:512], start=True, stop=True)
                        nc.tensor.matmul(out=ps1[:, 512:WAUG1], lhsT=lhs,
                                         rhs=w1aug_s[:, 512:WAUG1], start=True, stop=False)
                        nc.tensor.matmul(out=ps1[:, HF + F:WAUG1],
                                         lhsT=s01t_t, rhs=adst1_s[:, b, :],
                                         start=False, stop=True)
                        exv = p1.tile([128, EXW], F16, tag="exv")
                        e1 = p1.tile([128, 2 * H], F16, tag="e1")
                        nc.scalar.activation(out=e1[:, 0:H], in_=ps1[:, HF + F:WAUG1],
                                             func=AF.Exp)
                        nc.scalar.activation(out=e1[:, H:2 * H], in_=ps1[:, HF + F:WAUG1],
                                             func=AF.Exp, scale=0.2)
                        nc.vector.tensor_tensor(out=exv[:, HF + F:EXW], in0=e1[:, 0:H],
                                                in1=e1[:, H:2 * H], op=ALU.max)
                        nc.scalar.activation(out=exv[:, HF:HF + F], in_=ps1[:, HF:HF + F],
                                             func=AF.Copy,
                                             scale=enrm_s[:, t0 + k:t0 + k + 1])
                        nc.vector.tensor_tensor(
                            out=exv[:, 0:HF].rearrange("p (f h) -> p f h", f=F),
                            in0=ps1[:, 0:HF].rearrange("p (f h) -> p f h", f=F),
                            in1=exv[:, None, HF + F:EXW].to_broadcast([128, F, H]),
                            op=ALU.mult)
                        nc.tensor.matmul(out=ps_out[:, 0:512], lhsT=s01_t,
                                         rhs=exv[:, 0:512], start=(k == 0), stop=(k == TPB - 1))
                        nc.tensor.matmul(out=ps_out[:, 512:EXW], lhsT=s01_t,
                                         rhs=exv[:, 512:EXW], start=(k == 0), stop=(k == TPB - 1))
                    rec = p1.tile([128, H], F32, tag="rec")
                    nc.vector.reciprocal(out=rec[:], in_=ps_out[:, HF + F:EXW])
                    nc.vector.tensor_tensor(
                        out=x1loc_s[:, b, :].rearrange("p (f h) -> p f h", f=F),
                        in0=ps_out[:, 0:HF].rearrange("p (f h) -> p f h", f=F),
                        in1=rec[:, None, :].to_broadcast([128, F, H]),
                        op=ALU.mult)
                    nc.vector.tensor_copy(out=agg1_s[:, b, :], in_=ps_out[:, HF:HF + F])

            # ---------------- phase 2: comb table build + collective + conv
            with tc.tile_pool(name="p2", bufs=2) as p2:
              with tc.tile_pool(name="p2sa", bufs=2, space="PSUM") as p2s, \
                   tc.tile_pool(name="p2ta", bufs=2, space="PSUM") as p2t:
                x1t_s = p2.tile([128, 7, NPC], F16, tag="x1t", bufs=1)
                nc.gpsimd.memset(x1t_s[:], 0.0)
                for b in range(NBLK):
                    for fb in range(7):
                        c0, c1 = 128 * fb, min(128 * (fb + 1), HF)
                        psT = p2t.tile([128, 128], F16, space="PSUM", tag="psT")
                        nc.tensor.transpose(out=psT[:c1 - c0, :],
                                            in_=x1loc_s[:, b, c0:c1],
                                            identity=ident_s[:])
                        nc.vector.tensor_copy(
                            out=x1t_s[0:c1 - c0, fb, 128 * b:128 * (b + 1)],
                            in_=psT[:c1 - c0, :])
                for b in range(NBLK):
                    combst = combst2[:, b % 2, :]
                    # x2 for block b
                    psT = p2t.tile([128, 128], F16, space="PSUM", tag="psT")
                    nc.tensor.transpose(out=psT[:F, :], in_=agg1_s[:, b, :],
                                        identity=ident_s[:])
                    x2lt = p2.tile([128, 128], F16, tag="x2lt")
                    nc.vector.tensor_scalar(out=x2lt[:F, :], in0=psT[:F, :],
                                            scalar1=bgcnc_s[:F, :], scalar2=None,
                                            op0=ALU.add)
                    ps_x2 = p2s.tile([128, F], F32, space="PSUM", tag="psx2")
                    nc.tensor.matmul(out=ps_x2[:], lhsT=x2lt[:F, :], rhs=wgcn_s[:F, :],
                                     start=True, stop=True)
                    nc.vector.tensor_copy(out=combst[:, HF:HF + F], in_=ps_x2[:])
                    # T2 for block b
                    ps_t2 = p2s.tile([128, W2AUG], F32, space="PSUM", tag="pst2")
                    for k in range(7):
                        nc.tensor.matmul(out=ps_t2[:, 0:512],
                                         lhsT=x1t_s[:, k, 128 * b:128 * (b + 1)],
                                         rhs=w2aug_s[:, k, 0:512], start=(k == 0), stop=False)
                        nc.tensor.matmul(out=ps_t2[:, 512:W2AUG],
                                         lhsT=x1t_s[:, k, 128 * b:128 * (b + 1)],
                                         rhs=w2aug_s[:, k, 512:W2AUG], start=(k == 0), stop=False)
                    nc.tensor.matmul(out=ps_t2[:, 0:512], lhsT=ones_s[:],
                                     rhs=c2_s[:, 0:512], start=False, stop=True)
                    nc.tensor.matmul(out=ps_t2[:, 512:W2AUG], lhsT=ones_s[:],
                                     rhs=c2_s[:, 512:W2AUG], start=False, stop=True)
                    nc.vector.tensor_copy(out=combst[:, 0:HF], in_=ps_t2[:, 0:HF])
                    nc.vector.tensor_copy(out=combst[:, HF + F:WAUG1],
                                          in_=ps_t2[:, HF:HF + H])
                    nc.vector.tensor_copy(out=adst2_s[:, b, :], in_=ps_t2[:, HF + H:W2AUG])
                    nc.sync.dma_start(out=comb_shard[128 * b:128 * (b + 1), :],
                                      in_=combst)
                nc.gpsimd.collective_compute(
                    "AllGather", ALU.bypass, replica_groups=RG,
                    ins=[comb_shard[:]], outs=[comb_full[:]])

              with tc.tile_pool(name="p2sc", bufs=1, space="PSUM") as p2s, \
                   tc.tile_pool(name="p2tc", bufs=2, space="PSUM") as p2t:
                # conv branch overlaps the collective
                twin_s = p2.tile([32, GPC, 608], F16, tag="twin", bufs=1)
                nc.scalar.dma_start(out=twin_s[:], in_=twin_d[:])
                ps_ya = p2s.tile([GPC, 512], F32, space="PSUM", tag="psya")
                ps_yb = p2s.tile([GPC, 96], F32, space="PSUM", tag="psyb")
                for g in range(GPC):
                    nc.tensor.matmul(out=ps_ya[:], lhsT=wsel_s[:, g, :],
                                     rhs=twin_s[:, g, 0:512], start=(g == 0), stop=(g == GPC - 1))
                    nc.tensor.matmul(out=ps_yb[:], lhsT=wsel_s[:, g, :],
                                     rhs=twin_s[:, g, 512:608], start=(g == 0), stop=(g == GPC - 1))
                y_s = p2.tile([GPC, 608], F16, tag="ys")
                nc.vector.tensor_scalar(out=y_s[:, 0:512], in0=ps_ya[:],
                                        scalar1=bconv_s[:], scalar2=0.0,
                                        op0=ALU.add, op1=ALU.max)
                nc.vector.tensor_scalar(out=y_s[:, 512:608], in0=ps_yb[:],
                                        scalar1=bconv_s[:], scalar2=0.0,
                                        op0=ALU.add, op1=ALU.max)
                yt_s = pp.tile([128, 5, GPC], F16, tag="yt")
                nc.vector.memset(yt_s[:], 0.0)
                for i in range(5):
                    c0, c1 = 128 * i, min(128 * (i + 1), 608)
                    psT = p2t.tile([128, 128], F16, space="PSUM", tag="psT")
                    nc.tensor.transpose(out=psT[:c1 - c0, :GPC], in_=y_s[:, c0:c1],
                                        identity=ident_s[:GPC, :GPC])
                    nc.vector.tensor_copy(out=yt_s[0:c1 - c0, i, :], in_=psT[:c1 - c0, :GPC])
                ps_xt = p2s.tile([GPC, 256], F32, space="PSUM", tag="psxt")
                for i in range(5):
                    nc.tensor.matmul(out=ps_xt[:], lhsT=yt_s[:, i, :], rhs=wxt_s[:, i, :],
                                     start=(i == 0), stop=(i == 4))
                xt_s = p2.tile([GPC, 256], F16, tag="xts")
                nc.vector.tensor_tensor(out=xt_s[:], in0=ps_xt[:],
                                        in1=bias_s['bxt'][:], op=ALU.add)
                xtT_s = pp.tile([128, 2, GPC], F16, tag="xtT")
                for i in range(2):
                    psT = p2t.tile([128, 128], F16, space="PSUM", tag="psT")
                    nc.tensor.transpose(out=psT[:, :GPC], in_=xt_s[:, 128 * i:128 * (i + 1)],
                                        identity=ident_s[:GPC, :GPC])
                    nc.vector.tensor_copy(out=xtT_s[:, i, :], in_=psT[:, :GPC])

            # ---------------- phase 3: GAT2 + GCN2 (+ inline mean pool)
            with tc.tile_pool(name="p3", bufs=3) as p3, \
                 tc.tile_pool(name="p3g", bufs=2) as p3g, \
                 tc.tile_pool(name="p3s", bufs=3, space="PSUM") as p3s, \
                 tc.tile_pool(name="p3acc", bufs=1, space="PSUM") as p3acc, \
                 tc.tile_pool(name="p3m", bufs=1, space="PSUM") as p3m:
                ps_m = p3m.tile([GPC, HF + F], F32, space="PSUM", tag="psm", name="psm")[:]
                for b in range(NBLK):
                    TPB = TPBb[b]
                    EPB = TPB * 128
                    t0 = toff[b]
                    xfst = xfst2[:, b % 2, :]
                    v2g = p3g.tile([128, TPB, CROW], F16, tag="v2g")
                    nc.gpsimd.dma_gather(
                        out_ap=v2g[:], in_ap=comb_full[:],
                        idxs_ap=ixt2_s[:, t0 * 8:(t0 + TPB) * 8],
                        num_idxs=EPB, num_idxs_reg=EPB, elem_size=CROW,
                        single_packet=False)
                    s8_b = p3g.tile([128, TPB, 256], F8, tag="s8b3")
                    nc.sync.dma_start(out=s8_b[:], in_=s8_d[:, t0 * 256:(t0 + TPB) * 256])
                    ps_out = p3acc.tile([128, EXW], F32, space="PSUM", tag="psout3", name="psout3")[:]
                    for k in range(TPB):
                        s01_t = s8_b[:, k, 0:128]
                        s01t_t = s8_b[:, k, 128:256]
                        ps_l = p3s.tile([128, H], F32, space="PSUM", tag="psl")
                        nc.tensor.matmul(out=ps_l[:], lhsT=s01t_t,
                                         rhs=adst2_s[:, b, :], start=True, stop=False)
                        nc.tensor.matmul(out=ps_l[:], lhsT=ident_s[:],
                                         rhs=v2g[:, k, HF + F:WAUG1], start=False, stop=True)
                        exv = p3.tile([128, EXW], F16, tag="exv3")
                        e1 = p3.tile([128, 2 * H], F16, tag="e13")
                        nc.scalar.activation(out=e1[:, 0:H], in_=ps_l[:], func=AF.Exp)
                        nc.scalar.activation(out=e1[:, H:2 * H], in_=ps_l[:],
                                             func=AF.Exp, scale=0.2)
                        nc.vector.tensor_tensor(out=exv[:, HF + F:EXW], in0=e1[:, 0:H],
                                                in1=e1[:, H:2 * H], op=ALU.max)
                        nc.scalar.activation(out=exv[:, HF:HF + F], in_=v2g[:, k, HF:HF + F],
                                             func=AF.Copy,
                                             scale=enrm_s[:, t0 + k:t0 + k + 1])
                        nc.vector.tensor_tensor(
                            out=exv[:, 0:HF].rearrange("p (f h) -> p f h", f=F),
                            in0=v2g[:, k, 0:HF].rearrange("p (f h) -> p f h", f=F),
                            in1=exv[:, None, HF + F:EXW].to_broadcast([128, F, H]),
                            op=ALU.mult)
                        nc.tensor.matmul(out=ps_out[:, 0:512], lhsT=s01_t,
                                         rhs=exv[:, 0:512], start=(k == 0), stop=(k == TPB - 1))
                        nc.tensor.matmul(out=ps_out[:, 512:EXW], lhsT=s01_t,
                                         rhs=exv[:, 512:EXW], start=(k == 0), stop=(k == TPB - 1))
                    rec = p3.tile([128, H], F32, tag="rec3")
                    nc.vector.reciprocal(out=rec[:], in_=ps_out[:, HF + F:EXW])
                    u_s = p3.tile([128, HF], F16, tag="us")
                    nc.vector.tensor_tensor(
                        out=u_s[:].rearrange("p (f h) -> p f h", f=F),
                        in0=ps_out[:, 0:HF].rearrange("p (f h) -> p f h", f=F),
                        in1=rec[:, None, :].to_broadcast([128, F, H]),
                        op=ALU.mult)
                    v_s = p3.tile([128, HF], F16, tag="vs")
                    nc.vector.tensor_tensor(out=v_s[:], in0=u_s[:], in1=bg2_s[:],
                                            op=ALU.add)
                    nc.scalar.activation(out=xfst[:, 0:HF], in_=v_s[:], func=AF.Relu)
                    g2f = p3.tile([128, F], F32, tag="g2f")
                    nc.vector.tensor_tensor(out=g2f[:], in0=ps_out[:, HF:HF + F],
                                            in1=bgcnr_s[:], op=ALU.add)
                    nc.scalar.activation(out=xfst[:, HF:HF + F], in_=g2f[:], func=AF.Relu)
                    nc.scalar.activation(out=xfst[:, 896:896 + F], in_=g2f[:], func=AF.Relu)
                    nc.sync.dma_start(out=xf_dram[128 * b:128 * (b + 1), :], in_=xfst[:])
                    nc.tensor.matmul(out=ps_m[:, 0:512], lhsT=mmean_s[:, b, :],
                                     rhs=xfst[:, 0:512], start=(b == 0),
                                     stop=(b == NBLK - 1))
                    nc.tensor.matmul(out=ps_m[:, 512:HF + F], lhsT=mmean_s[:, b, :],
                                     rhs=xfst[:, 512:HF + F], start=(b == 0),
                                     stop=(b == NBLK - 1))
                mean_s = pp.tile([GPC, HF + F], F16, tag="means")
                nc.vector.tensor_copy(out=mean_s[:], in_=ps_m[:])

            # ---------------- phase 4: max pool + head
            with tc.tile_pool(name="p4", bufs=2) as p4:
              with tc.tile_pool(name="p4s", bufs=1, space="PSUM") as p4s:
                gmax1T = pp.tile([128, 7, GPC], F16, tag="gmax1T")
                nc.vector.memset(gmax1T[:], 0.0)
                gmax2T = pp.tile([128, 1, GPC], F16, tag="gmax2T")
                nc.vector.memset(gmax2T[:], 0.0)
                CH = GPC // 2
                for h in range(2):
                    slab = p4.tile([128, 7, CH * PW], F16, tag="slab")
                    nc.gpsimd.dma_gather(
                        out_ap=slab[:], in_ap=xf_dram[:, 0:CROW],
                        idxs_ap=ixp_s[:, h * (CH * PW // 16):(h + 1) * (CH * PW // 16)],
                        num_idxs=CH * PW, num_idxs_reg=CH * PW, elem_size=CROW,
                        elem_step=1024, transpose=True, single_packet=False)
                    slab2 = p4.tile([128, 1, CH * PW], F16, tag="slab2")
                    nc.gpsimd.dma_gather(
                        out_ap=slab2[:], in_ap=xf_dram[:, 896:1024],
                        idxs_ap=ixp_s[:, h * (CH * PW // 16):(h + 1) * (CH * PW // 16)],
                        num_idxs=CH * PW, num_idxs_reg=CH * PW, elem_size=XROW,
                        elem_step=1024, transpose=True, single_packet=False)
                    for j in range(7):
                        nc.vector.tensor_reduce(
                            out=gmax1T[:, j, h * CH:(h + 1) * CH],
                            in_=slab[:, j, :].rearrange("p (g w) -> p g w", g=CH),
                            op=ALU.max, axis=AX)
                    nc.vector.tensor_reduce(
                        out=gmax2T[:, 0, h * CH:(h + 1) * CH],
                        in_=slab2[:, 0, :].rearrange("p (g w) -> p g w", g=CH),
                        op=ALU.max, axis=AX)
              with tc.tile_pool(name="p4sh", bufs=1, space="PSUM") as p4s:
                gmean1T = pp.tile([128, 7, GPC], F16, tag="gmean1T")
                nc.vector.memset(gmean1T[:], 0.0)
                gmean2T = pp.tile([128, 1, GPC], F16, tag="gmean2T")
                nc.vector.memset(gmean2T[:], 0.0)
                for i in range(7):
                    c0, c1 = 128 * i, min(128 * (i + 1), HF)
                    psT = p4s.tile([128, 128], F16, space="PSUM", tag="psT4", bufs=2)
                    nc.tensor.transpose(out=psT[:c1 - c0, :GPC], in_=mean_s[:, c0:c1],
                                        identity=ident_s[:GPC, :GPC])
                    nc.vector.tensor_copy(out=gmean1T[0:c1 - c0, i, :], in_=psT[:c1 - c0, :GPC])
                psT = p4s.tile([128, 128], F16, space="PSUM", tag="psT4", bufs=2)
                nc.tensor.transpose(out=psT[:F, :GPC], in_=mean_s[:, HF:HF + F],
                                    identity=ident_s[:GPC, :GPC])
                nc.vector.tensor_copy(out=gmean2T[0:F, 0, :], in_=psT[:F, :GPC])

                def head_mm(ps, chunks, rhs_tile, nw):
                    n = len(chunks)
                    for i, ch in enumerate(chunks):
                        nc.tensor.matmul(out=ps[:], lhsT=ch, rhs=rhs_tile[:, i, :nw],
                                         start=(i == 0), stop=(i == n - 1))

                def bias_relu_T(ps, bias_ap, w, relu, nT, tagb):
                    zs = p4.tile([GPC, w], F16, tag="z" + tagb)
                    nc.vector.tensor_tensor(out=zs[:], in0=ps[:], in1=bias_ap, op=ALU.add)
                    if relu:
                        nc.vector.tensor_scalar(out=zs[:], in0=zs[:], scalar1=0.0,
                                                scalar2=None, op0=ALU.max)
                    zT = pp.tile([128, nT, GPC], F16, tag="zT" + tagb)
                    for i in range(nT):
                        psT2 = p4s.tile([128, 128], F16, space="PSUM", tag="psT4", bufs=2)
                        nc.tensor.transpose(out=psT2[:, :GPC], in_=zs[:, 128 * i:128 * (i + 1)],
                                            identity=ident_s[:GPC, :GPC])
                        nc.vector.tensor_copy(out=zT[:, i, :], in_=psT2[:, :GPC])
                    return zT

                ps_z1 = p4s.tile([GPC, 128], F32, space="PSUM", tag="psz1")
                head_mm(ps_z1, [gmax1T[:, j, :] for j in range(7)]
                        + [gmean1T[:, j, :] for j in range(7)], wfg1_s, 128)
                z1T = bias_relu_T(ps_z1, bias_s['bfg1'][:], 128, True, 1, "1")
                ps_z2 = p4s.tile([GPC, 128], F32, space="PSUM", tag="psz2")
                head_mm(ps_z2, [gmax2T[:, 0, :], gmean2T[:, 0, :]], wfg2_s, 128)
                z2T = bias_relu_T(ps_z2, bias_s['bfg2'][:], 128, True, 1, "2")
                ps_h1 = p4s.tile([GPC, 512], F32, space="PSUM", tag="psh1")
                head_mm(ps_h1, [z1T[:, 0, :], z2T[:, 0, :], xtT_s[:, 0, :], xtT_s[:, 1, :]],
                        w1_s, 512)
                h1T = bias_relu_T(ps_h1, bias_s['b1'][:], 512, True, 4, "h1")
                ps_h2 = p4s.tile([GPC, 256], F32, space="PSUM", tag="psh2")
                head_mm(ps_h2, [h1T[:, i, :] for i in range(4)], w2_s, 256)
                h2T = bias_relu_T(ps_h2, bias_s['b2'][:], 256, True, 2, "h2")
                ps_o = p4s.tile([GPC, 1], F32, space="PSUM", tag="pso")
                head_mm(ps_o, [h2T[:, i, :] for i in range(2)], wo_s, 1)
                o_s = p4.tile([GPC, 1], F32, tag="os")
                nc.vector.tensor_scalar(out=o_s[:], in0=ps_o[:], scalar1=bo_s[:],
                                        scalar2=None, op0=ALU.add)
                nc.sync.dma_start(out=out_d[:], in_=o_s[:])

    nc.compile()
    return nc


def build_in_maps(nc, shared, cores):
    declared = set()
    import concourse.mybir as _mb
    for alloc in nc.m.functions[0].allocations:
        if isinstance(alloc, _mb.MemoryLocationSet) and alloc.kind == "ExternalInput":
            declared.add(alloc.memorylocations[0].name)
    in_maps = []
    for c in range(8):
        m = dict(shared)
        m.update(cores[c])
        in_maps.append({k: np.ascontiguousarray(v) for k, v in m.items()
                        if k in declared})
    return in_maps


_CACHE = {}


def run_device(inputs):
    meta, shared, cores = prep(**inputs)
    key = (meta['NBLK'], meta['TPBb'], meta['PW'])
    if key not in _CACHE:
        _CACHE[key] = build(meta)
    nc = _CACHE[key]
    in_maps = build_in_maps(nc, shared, cores)
    res = run_bass_kernel_spmd(nc, in_maps, core_ids=list(range(8)))
    out = np.concatenate([res.results[c]['out'] for c in range(8)], axis=0)
    return out.astype(np.float32)


def kernel(**inputs):
    return run_device(inputs)
